# revision 1
# baseline (speedup 1.0000x reference)
"""GAT 3-layer kernel for 8 TRN2 NeuronCores (slot-major edge-parallel design).

Sharding: dst nodes packed into 392 blocks of 128 slots (one slot = one SBUF
partition), blocks dealt to 8 cores x 49 positions. Edges live on the free dim
of each block (j-slabs), split lo/hi by source-table row (<32768 vs >=) so the
int16 dma_gather indices stay in range. Node features/tables are all-gathered
per layer (chunked, overlapped with the node phase); everything else is local.
"""
import numpy as np

N_NODES = 50000
E_EDGES = 800000
IN_FEATS = 256
HID = 64
HEADS = 4
NCLS = 40
NEG_SLOPE = 0.2

NCORES = 8
NPOS = 49                    # blocks per core
NPA = 32                     # positions in table A (rest in table B)
SLAB_A = NPA * 128           # 4096 rows/core in table A
SLAB_B = (NPOS - NPA) * 128  # 2176 rows/core in table B
ROWS_A = NCORES * SLAB_A     # 32768
ROWS_B = NCORES * SLAB_B     # 17408
A_SIZE = ROWS_A - 8          # A-region real-node capacity (8 reserved slots)
DUMMY_A_ROW = ROWS_A - 1     # (core 7, pos 31, slot 127)
DUMMY_B_ROW = ROWS_B - 1     # (core 7, pos 48, slot 127)
RES_POS = (NPA - 1, NPOS - 1)  # slot 127 reserved on every core

# AllGather chunk boundaries (position ranges); A chunks then B chunks.
# Tables are laid out chunk-major ([chunk][core][rows-in-chunk]) so each
# chunk's AllGather writes one contiguous range.
ACH = [(0, 32)]
BCH = [(32, 49)]


def _srow_of(core, pos, slot):
    """Table row for a node at (core, pos, slot) under chunk-major layout."""
    pos = np.asarray(pos)
    out = np.zeros(pos.shape, np.int64)
    base = 0
    for (lo, hi) in ACH:
        nrow = (hi - lo) * 128
        m = (pos >= lo) & (pos < hi)
        out[m] = base + core[m] * nrow + (pos[m] - lo) * 128 + slot[m]
        base += NCORES * nrow
    base = 0
    for (lo, hi) in BCH:
        nrow = (hi - lo) * 128
        m = (pos >= lo) & (pos < hi)
        out[m] = base + core[m] * nrow + (pos[m] - lo) * 128 + slot[m]
        base += NCORES * nrow
    return out


def _pack_nodes(src, dst):
    """Assign each node a (core, pos, slot); A-set = ids < A_SIZE -> pos < NPA.

    2D-banded packing: band by one in-degree coordinate, sort by the other
    within each band, so each 1024-node position block is homogeneous in
    (n_a, n_b) and per-block maxima (the slab padding) stay small."""
    deg = np.bincount(dst, minlength=N_NODES)
    n_a = np.bincount(dst[src < A_SIZE], minlength=N_NODES)
    n_b = deg - n_a

    ids = np.arange(N_NODES)
    a_ids = ids[:A_SIZE]
    b_ids = ids[A_SIZE:]

    def banded(idset, pri, sec, G):
        o = idset[np.argsort(-pri[idset], kind='stable')]
        out = []
        for i in range(0, len(o), 1024 * G):
            band = o[i:i + 1024 * G]
            out.append(band[np.argsort(-sec[band], kind='stable')])
        return np.concatenate(out)

    def chunk_cost(order, npos):
        tot, k = 0, 0
        for p in range(npos):
            take = order[k:k + 1024]
            k += len(take)
            tot += max(n_a[take].max(), 1) + max(n_b[take].max(), 1)
        return tot

    def best(idset, npos):
        cands = []
        for G in (2, 3, 4, 5, 6):
            for pri, sec in ((n_a, n_b), (n_b, n_a)):
                o = banded(idset, pri, sec, G)
                cands.append((chunk_cost(o, npos), o))
        cands.sort(key=lambda t: t[0])
        return cands[0][1]

    a_sorted = best(a_ids, NPA)
    b_sorted = best(b_ids, NPOS - NPA)

    core_of = np.full(N_NODES, -1, np.int32)
    pos_of = np.full(N_NODES, -1, np.int32)
    slot_of = np.full(N_NODES, -1, np.int32)

    def fill(sorted_ids, pos0):
        k = 0
        pos, c = pos0, 0
        while k < len(sorted_ids):
            cap = 127 if pos in RES_POS else 128
            take = sorted_ids[k:k + cap]
            k += len(take)
            core_of[take] = c
            pos_of[take] = pos
            slot_of[take] = np.arange(len(take))
            c += 1
            if c == NCORES:
                c = 0
                pos += 1

    fill(a_sorted, 0)
    fill(b_sorted, NPA)
    return core_of, pos_of, slot_of


def _build_grids(src, dst, core_of, pos_of, slot_of):
    """Per-core wrapped int16 idx arrays + per-position J_a/J_b schedules."""
    in_b = (pos_of[src] >= NPA).astype(np.int64)
    srow = _srow_of(core_of[src], pos_of[src], slot_of[src])
    dcore = core_of[dst]
    dpos = pos_of[dst]
    dslot = slot_of[dst]
    drow = dcore * (NPOS * 128) + dpos * 128 + dslot

    order = np.lexsort((srow, in_b, drow))
    gkey = drow[order] * 2 + in_b[order]
    newgrp = np.concatenate([[True], gkey[1:] != gkey[:-1]])
    gstart = np.maximum.accumulate(np.where(newgrp, np.arange(E_EDGES), 0))
    rank = np.arange(E_EDGES) - gstart
    j_in = np.empty(E_EDGES, np.int64)
    j_in[order] = rank

    na_e = np.where(in_b == 0, j_in + 1, 0)
    nb_e = np.where(in_b == 1, j_in + 1, 0)
    J_a = np.zeros(NPOS, np.int64)
    J_b = np.zeros(NPOS, np.int64)
    np.maximum.at(J_a, dpos, na_e)
    np.maximum.at(J_b, dpos, nb_e)
    J_a = np.maximum(J_a, 1)
    J_b = np.maximum(J_b, 1)

    grids = [[np.concatenate([
                np.full((J_a[p], 128), DUMMY_A_ROW, np.int64),
                np.full((J_b[p], 128), DUMMY_B_ROW, np.int64)])
              for p in range(NPOS)] for c in range(NCORES)]
    j_eff = np.where(in_b == 0, j_in, J_a[dpos] + j_in)
    for e in range(E_EDGES):
        grids[dcore[e]][dpos[e]][j_eff[e], dslot[e]] = srow[e]

    def wrap(grid):
        flat = grid.reshape(-1)
        w = flat.reshape(-1, 8, 16).transpose(2, 0, 1).reshape(16, -1)
        return np.tile(w, (8, 1)).astype(np.int16)

    idxT = []
    colbase = []
    for c in range(NCORES):
        parts = []
        cb = []
        col = 0
        for p in range(NPOS):
            a_w = wrap(grids[c][p][:J_a[p]])
            b_w = wrap(grids[c][p][J_a[p]:])
            cb.append((col, col + a_w.shape[1]))
            col += a_w.shape[1] + b_w.shape[1]
            parts.append(a_w)
            parts.append(b_w)
        idxT.append(np.concatenate(parts, axis=1))
        colbase = cb
    return idxT, J_a.tolist(), J_b.tolist(), colbase


def _fold_weights(W, al, ar):
    """[W | A | B] with A[k,h]=sum_d W[k,h*D+d]*al[h,d] (el), B likewise (er)."""
    H, D = al.shape
    Wr = W.reshape(W.shape[0], H, D)
    A = np.einsum('khd,hd->kh', Wr, al)
    B = np.einsum('khd,hd->kh', Wr, ar)
    return np.concatenate([W, A, B], axis=1).astype(np.float16)


def _build_program(J_a, J_b, colbase):
    import concourse.bacc as bacc
    import concourse.bass as bass
    import concourse.mybir as mybir
    from concourse.tile import TileContext

    f16 = mybir.dt.float16
    f32 = mybir.dt.float32
    AF = mybir.ActivationFunctionType
    OP = mybir.AluOpType

    TCOLS = [384, 384, 256]      # fp16 cols per table row per layer (stride)
    GCOLS = [256, 256, 160]      # ft cols per layer
    CCOLS = [264, 264, 168]      # node-phase out cols (ft + el4 + er4)
    TCC = [264, 264, 168]        # compact staged cols (ft + el4-as-f32)
    TOTC = colbase[-1][1] + J_b[-1] * 8
    Jt = [J_a[p] + J_b[p] for p in range(NPOS)]

    # g-tile size classes: top-2 positions -> 'gl' (bufs=2), rest 'gs'
    s_sorted = sorted(Jt, reverse=True)
    GL_MIN = s_sorted[2] + 1        # positions with Jt above this go to gl
    gl_bytes = 2 * s_sorted[0] * 384 * 2
    gs4_bytes = 4 * s_sorted[2] * 384 * 2
    gs_bufs = 4 if gl_bytes + gs4_bytes <= 135 * 1024 else 3

    nc = bacc.Bacc("TRN2", num_devices=NCORES, num_swdge_queues=4)
    featT = nc.dram_tensor("featT", [256, NPOS * 128], f16, kind="ExternalInput")
    idxT = nc.dram_tensor("idxT", [128, TOTC], mybir.dt.int16, kind="ExternalInput")
    Wes = [nc.dram_tensor(f"W{l}e", [256, CCOLS[l]], f16, kind="ExternalInput")
           for l in range(3)]
    out_d = nc.dram_tensor("out", [NPOS * 128, NCLS], f32, kind="ExternalOutput")

    agins = []   # per layer: list of (pos_lo, pos_hi, is_b, row_base, tensor)
    for l in range(3):
        chunks = []
        base = 0
        for (lo, hi) in ACH:
            t = nc.dram_tensor(f"aga{l}_{lo}", [(hi - lo) * 128, TCOLS[l]],
                               f16, kind="Internal")
            chunks.append((lo, hi, False, base, t))
            base += NCORES * (hi - lo) * 128
        base = 0
        for (lo, hi) in BCH:
            t = nc.dram_tensor(f"agb{l}_{lo}", [(hi - lo) * 128, TCOLS[l]],
                               f16, kind="Internal")
            chunks.append((lo, hi, True, base, t))
            base += NCORES * (hi - lo) * 128
        agins.append(chunks)
    table_a = [nc.dram_tensor(f"tablea{l}", [ROWS_A, TCOLS[l]], f16,
                              kind="Internal", addr_space="Shared")
               for l in range(3)]
    table_b = [nc.dram_tensor(f"tableb{l}", [ROWS_B, TCOLS[l]], f16,
                              kind="Internal", addr_space="Shared")
               for l in range(3)]

    qn = [0]

    def next_q():
        qn[0] = (qn[0] + 1) % 4
        return qn[0]

    with TileContext(nc) as tc:
        with tc.tile_pool(name="resident", bufs=1) as rp, \
             tc.tile_pool(name="work", bufs=3) as wp, \
             tc.tile_pool(name="gather", bufs=2) as gp, \
             tc.tile_pool(name="nps", bufs=3, space="PSUM") as nps, \
             tc.tile_pool(name="tps", bufs=4, space="PSUM") as tps:

            ia = rp.tile([128, TOTC], mybir.dt.int16)
            nc.sync.dma_start(ia[:], idxT[:])
            NP128 = NPOS * 128
            hTbig = rp.tile([128, 2 * NP128], f16)
            nc.sync.dma_start(hTbig[:, :NP128], featT[0:128, :])
            nc.sync.dma_start(hTbig[:, NP128:], featT[128:256, :])
            hT = [[bass.AP(hTbig.tensor,
                           hTbig[:].offset + k * NP128 + p * 128,
                           [hTbig[:].ap[0], [1, 128]])
                   for p in range(NPOS)] for k in range(2)]
            er_own = [rp.tile([128, 4], f32, tag=f"er{p}", name=f"er{p}")
                      for p in range(NPOS)]
            Wt = [rp.tile([128, 2, CCOLS[l]], f16, tag=f"Wt{l}", name=f"Wt{l}")
                  for l in range(3)]
            for l in range(3):
                nc.sync.dma_start(
                    Wt[l][:], Wes[l][:].rearrange("(k p) n -> p k n", k=2))
            # fp32 identity for PE transpose
            colv = rp.tile([128, 128], mybir.dt.int32)
            nc.gpsimd.iota(colv[:], [[1, 128]], base=0, channel_multiplier=0)
            rowv = rp.tile([128, 1], mybir.dt.int32)
            nc.gpsimd.iota(rowv[:], [[0, 1]], base=0, channel_multiplier=1)
            row_b = bass.AP(rowv.tensor, rowv[:].offset,
                            [rowv[:].ap[0], [0, 128]])
            identf = rp.tile([128, 128], f32)
            nc.vector.tensor_tensor(identf[:], colv[:], row_b, OP.is_equal)
            # -80 at partition 127, 0 elsewhere (dummy-row el marker)
            dmask = rp.tile([128, 1], f32)
            nc.vector.tensor_scalar(dmask[:], rowv[:], 127, -80.0,
                                    OP.is_equal, OP.mult)

            def node_phase(l, p):
                GC, CC = GCOLS[l], CCOLS[l]
                ps = nps.tile([128, CC], f32, tag="nodeps")
                for k in range(2):
                    nc.tensor.matmul(
                        ps[:], hT[k][p],
                        Wt[l][:].rearrange("p k n -> k p n")[k],
                        start=(k == 0), stop=(k == 1))
                nc.vector.tensor_copy(er_own[p][:], ps[:, GC + 4:GC + 8])
                stage = wp.tile([128, TCOLS[l]], f16, tag="stage")
                nc.vector.tensor_copy(stage[:, :GC], ps[:, :GC])
                st32 = stage[:].bitcast(f32)
                if p in RES_POS:
                    dm_b = bass.AP(dmask.tensor, dmask[:].offset,
                                   [dmask[:].ap[0], [0, 4]])
                    nc.vector.tensor_tensor(st32[:, GC // 2:GC // 2 + 4],
                                            ps[:, GC:GC + 4], dm_b, OP.add)
                else:
                    nc.vector.tensor_copy(st32[:, GC // 2:GC // 2 + 4],
                                          ps[:, GC:GC + 4])
                # stage -> the agin chunk containing position p (scalar-queue
                # HWDGE so the sync queue stays free)
                for (lo, hi, is_b, rb, t) in agins[l]:
                    if lo <= p < hi:
                        nc.scalar.dma_start(
                            t[(p - lo) * 128:(p - lo + 1) * 128, :], stage[:])
                # fire the collective for any chunk that just completed.
                # All of this sweep's gathers were emitted before any of
                # these, so the collective only delays the NEXT sweep's
                # gathers (which depend on its output anyway).
                for (lo, hi, is_b, rb, t) in agins[l]:
                    if p == hi - 1:
                        tab = table_b[l] if is_b else table_a[l]
                        nrow = (hi - lo) * 128
                        nc.gpsimd.collective_compute(
                            "AllGather", OP.bypass,
                            replica_groups=[list(range(NCORES))],
                            ins=[t[:].opt()],
                            outs=[tab[rb:rb + NCORES * nrow, :].opt()])

            g_tiles = {}

            def edge_gather(l, p):
                GC, TC = GCOLS[l], TCOLS[l]
                Jl, Jh = J_a[p], J_b[p]
                Jtp = Jl + Jh
                locol, hicol = colbase[p]
                tag = "gl" if Jtp >= GL_MIN else "gs"
                nb = 2 if tag == "gl" else gs_bufs
                g = gp.tile([128, Jtp, 384], f16, tag=tag, bufs=nb, name="g")
                g_tiles[p] = g
                nc.gpsimd.dma_gather(
                    bass.AP(g.tensor, g[:].offset,
                            [g[:].ap[0], [TC, Jl], [1, TC]]),
                    table_a[l][:, :],
                    ia[:, locol:locol + Jl * 8], Jl * 128, Jl * 128, TC,
                    single_packet=False, queue_num=next_q())
                nc.gpsimd.dma_gather(
                    bass.AP(g.tensor, g[:].offset + Jl * TC,
                            [g[:].ap[0], [TC, Jh], [1, TC]]),
                    table_b[l][:, :],
                    ia[:, hicol:hicol + Jh * 8], Jh * 128, Jh * 128, TC,
                    single_packet=False, queue_num=next_q())

            def edge_phase(l, p):
                GC, TC = GCOLS[l], TCOLS[l]
                Jl, Jh = J_a[p], J_b[p]
                Jtp = Jl + Jh
                g = g_tiles.pop(p)
                # e = el + er_bcast   (h-major [128, 4, Jt] layout)
                elv = bass.AP(g.tensor, g[:].offset,
                              [g[:].ap[0], [TC, Jtp], [1, TC]]).bitcast(f32)
                el_hm = bass.AP(elv.tensor, elv.offset + GC // 2,
                                [elv.ap[0], [1, 4], [TC // 2, Jtp]])
                e_t = wp.tile([128, 4, Jtp], f32, tag="e")
                er_b = bass.AP(er_own[p].tensor, er_own[p][:].offset,
                               [er_own[p][:].ap[0], [1, 4], [0, Jtp]])
                nc.vector.tensor_tensor(e_t[:], el_hm, er_b, OP.add)
                # ex = exp(lrelu(e)) = max(exp(e), exp(0.2 e))
                ex1 = wp.tile([128, 4, Jtp], f32, tag="ex1")
                nc.scalar.activation(ex1[:], e_t[:], AF.Exp)
                ex2 = wp.tile([128, 4, Jtp], f32, tag="ex2")
                nc.scalar.activation(ex2[:], e_t[:], AF.Exp, scale=NEG_SLOPE)
                nc.vector.tensor_tensor(ex1[:], ex1[:], ex2[:], OP.max)
                # denom over j (inner-contiguous); rd = 1/max(denom, 1e-9)
                den = wp.tile([128, 4], f32, tag="den")
                nc.vector.tensor_reduce(den[:, :, None], ex1[:], op=OP.add,
                                        axis=mybir.AxisListType.X)
                nc.vector.tensor_scalar_max(den[:], den[:], 1e-9)
                rd = wp.tile([128, 4], f32, tag="rd")
                nc.vector.reciprocal(rd[:], den[:])
                # alpha = ex * rd_bcast  (fp16, h-major)
                alpha = wp.tile([128, 4, Jtp], f16, tag="alpha")
                rd_b = bass.AP(rd.tensor, rd[:].offset,
                               [rd[:].ap[0], [1, 4], [0, Jtp]])
                nc.vector.tensor_tensor(alpha[:], ex1[:], rd_b, OP.mult)
                # msg = alpha * ft, in place into g's ft cols
                D = GC // 4
                al_b = bass.AP(alpha.tensor, alpha[:].offset,
                               [alpha[:].ap[0], [1, Jtp], [Jtp, 4], [0, D]])
                ft4 = bass.AP(g.tensor, g[:].offset,
                              [g[:].ap[0], [TC, Jtp], [D, 4], [1, D]])
                nc.vector.tensor_tensor(ft4, ft4, al_b, OP.mult)
                # out_row = sum_j msg  (halving tree over j, in place)
                ro = wp.tile([128, GC], f32, tag="ro")

                def jsl(j0, cnt):
                    return bass.AP(g.tensor, g[:].offset + j0 * TC,
                                   [g[:].ap[0], [TC, cnt], [1, GC]])

                n = Jtp
                while n > 2:
                    h = n // 2
                    nc.vector.tensor_tensor(jsl(0, h), jsl(0, h),
                                            jsl(n - h, h), OP.add)
                    n -= h
                if n == 2:
                    nc.vector.tensor_tensor(ro[:], jsl(0, 1), jsl(1, 1),
                                            OP.add)
                else:
                    nc.vector.tensor_copy(ro[:], jsl(0, 1))
                if l < 2:
                    # h = elu(ro) = max(ro, min(exp(ro),1)-1); -> fp16 -> hT
                    ev = wp.tile([128, GC], f32, tag="ev")
                    nc.scalar.activation(ev[:], ro[:], AF.Exp)
                    nc.vector.tensor_scalar(ev[:], ev[:], 1.0, -1.0,
                                            OP.min, OP.add)
                    nc.vector.tensor_tensor(ro[:], ro[:], ev[:], OP.max)
                    for k in range(2):
                        tp = tps.tile([128, 128], f32, tag="trps")
                        nc.tensor.transpose(
                            tp[:], ro[:, k * 128:(k + 1) * 128], identf[:])
                        nc.vector.tensor_copy(hT[k][p], tp[:])
                else:
                    # logits = mean over heads; log_softmax
                    z = wp.tile([128, NCLS], f32, tag="z")
                    ro_h = bass.AP(ro.tensor, ro[:].offset,
                                   [ro[:].ap[0], [1, NCLS], [NCLS, 4]])
                    z_v = bass.AP(z.tensor, z[:].offset,
                                  [z[:].ap[0], [1, NCLS], [0, 1]])
                    nc.vector.tensor_reduce(z_v, ro_h, op=OP.add,
                                            axis=mybir.AxisListType.X)
                    nc.vector.tensor_scalar_mul(z[:], z[:], 0.25)
                    m = wp.tile([128, 1], f32, tag="m")
                    nc.vector.tensor_reduce(m[:], z[:], op=OP.max,
                                            axis=mybir.AxisListType.X)
                    nm = wp.tile([128, 1], f32, tag="nm")
                    nc.vector.tensor_scalar_mul(nm[:], m[:], -1.0)
                    ez = wp.tile([128, NCLS], f32, tag="ez")
                    s = wp.tile([128, 1], f32, tag="s")
                    nc.scalar.activation(ez[:], z[:], AF.Exp, bias=nm[:],
                                         accum_out=s[:])
                    lns = wp.tile([128, 1], f32, tag="lns")
                    nc.scalar.activation(lns[:], s[:], AF.Ln)
                    b = wp.tile([128, 1], f32, tag="b")
                    nc.vector.tensor_tensor(b[:], m[:], lns[:], OP.add)
                    lp = wp.tile([128, NCLS], f32, tag="lp")
                    b_b = bass.AP(b.tensor, b[:].offset,
                                  [b[:].ap[0], [0, NCLS]])
                    nc.vector.tensor_tensor(lp[:], z[:], b_b, OP.subtract)
                    nc.sync.dma_start(out_d[p * 128:(p + 1) * 128, :], lp[:])

            # sweep 0: node phase of layer 0 (+ its chunked all-gathers)
            for p in range(NPOS):
                node_phase(0, p)
            # sweeps 1,2: edge(l-1) interleaved with node(l)
            for l in (1, 2):
                for p in range(NPOS):
                    edge_gather(l - 1, p)
                for p in range(NPOS):
                    edge_phase(l - 1, p)
                    node_phase(l, p)
            # sweep 3: edge phase of layer 2 -> logits
            for p in range(NPOS):
                edge_gather(2, p)
            for p in range(NPOS):
                edge_phase(2, p)
    nc.compile()
    return nc


def _install_trace_shim():
    """Provide antenv.axon_hooks (get/set NTFF profile hook) when absent."""
    import sys, types
    try:
        from antenv.axon_hooks import get_axon_ntff_profile_hook  # noqa
        return
    except ImportError:
        pass
    mod = types.ModuleType("antenv.axon_hooks")
    _hook = [None]
    mod.set_axon_ntff_profile_hook = lambda h: _hook.__setitem__(0, h)
    mod.get_axon_ntff_profile_hook = lambda: _hook[0]
    sys.modules["antenv.axon_hooks"] = mod
    import antenv
    antenv.axon_hooks = mod
    if "/root/.axon_site" not in sys.path:
        sys.path.insert(0, "/root/.axon_site")
    from trn_agent_boot.trn_boot import _ntff_profile_via_ctypes
    mod.set_axon_ntff_profile_hook(
        _ntff_profile_via_ctypes("/opt/axon/libaxon_pjrt.so"))


def kernel(features, src, dst, W0, al0, ar0, W1, al1, ar1, W2, al2, ar2):
    import sys, os
    for pth in ("/root/axon_fix", "/opt/trn_rl_repo"):
        if os.path.isdir(pth) and pth not in sys.path:
            sys.path.insert(0, pth)
    if os.environ.get("KERNEL_TRACE"):
        _install_trace_shim()
    from concourse import bass_utils

    src = np.asarray(src).astype(np.int64)
    dst = np.asarray(dst).astype(np.int64)
    features = np.asarray(features, np.float32)

    core_of, pos_of, slot_of = _pack_nodes(src, dst)
    idxT, J_a, J_b, colbase = _build_grids(src, dst, core_of, pos_of, slot_of)
    Wes = [_fold_weights(np.asarray(W0, np.float32), np.asarray(al0), np.asarray(ar0)),
           _fold_weights(np.asarray(W1, np.float32), np.asarray(al1), np.asarray(ar1)),
           _fold_weights(np.asarray(W2, np.float32), np.asarray(al2), np.asarray(ar2))]

    # per-core featT [256, SLAB] fp16 in slot order
    featTs = []
    for c in range(NCORES):
        ft = np.zeros((256, NPOS * 128), np.float16)
        mask = core_of == c
        ids = np.arange(N_NODES)[mask]
        cols = pos_of[ids] * 128 + slot_of[ids]
        ft[:, cols] = features[ids].T.astype(np.float16)
        featTs.append(ft)

    nc = _build_program(J_a, J_b, colbase)
    ins = [{"featT": featTs[c], "idxT": idxT[c],
            "W0e": Wes[0], "W1e": Wes[1], "W2e": Wes[2]}
           for c in range(NCORES)]
    res = bass_utils.run_bass_kernel_spmd(
        nc, ins, core_ids=list(range(NCORES)),
        trace=bool(os.environ.get("KERNEL_TRACE")))
    if os.environ.get("KERNEL_TRACE"):
        print("HW exec time:", res.exec_time_ns, "ns")
        kernel.last_exec_ns = res.exec_time_ns
        kernel.last_trace = res.instructions_and_trace

    out = np.empty((N_NODES, NCLS), np.float32)
    for c in range(NCORES):
        mask = core_of == c
        ids = np.arange(N_NODES)[mask]
        rows = pos_of[ids] * 128 + slot_of[ids]
        out[ids] = res.results[c]["out"][rows]
    return out



# revision 5
# speedup vs baseline: 1.0009x; 1.0009x over previous
"""GAT 3-layer kernel for 8 TRN2 NeuronCores (slot-major edge-parallel design).

Sharding: dst nodes packed into 392 blocks of 128 slots (one slot = one SBUF
partition), blocks dealt to 8 cores x 49 positions. Edges live on the free dim
of each block (j-slabs), split lo/hi by source-table row (<32768 vs >=) so the
int16 dma_gather indices stay in range. Node features/tables are all-gathered
per layer (chunked, overlapped with the node phase); everything else is local.
"""
import numpy as np

N_NODES = 50000
E_EDGES = 800000
IN_FEATS = 256
HID = 64
HEADS = 4
NCLS = 40
NEG_SLOPE = 0.2

NCORES = 8
NPOS = 49                    # blocks per core
NPA = 32                     # positions in table A (rest in table B)
SLAB_A = NPA * 128           # 4096 rows/core in table A
SLAB_B = (NPOS - NPA) * 128  # 2176 rows/core in table B
ROWS_A = NCORES * SLAB_A     # 32768
ROWS_B = NCORES * SLAB_B     # 17408
A_SIZE = ROWS_A - 8          # A-region real-node capacity (8 reserved slots)
DUMMY_A_ROW = ROWS_A - 1     # (core 7, pos 31, slot 127)
DUMMY_B_ROW = ROWS_B - 1     # (core 7, pos 48, slot 127)
RES_POS = (NPA - 1, NPOS - 1)  # slot 127 reserved on every core

# AllGather chunk boundaries (position ranges); A chunks then B chunks.
# Tables are laid out chunk-major ([chunk][core][rows-in-chunk]) so each
# chunk's AllGather writes one contiguous range. Finer chunks let the
# collectives start while the sweep is still running; the last chunk of
# each region is a single position so the AG gating the next sweep's
# gathers is tiny.
ACH = [(0, 16), (16, 28), (28, 31), (31, 32)]
BCH = [(32, 40), (40, 45), (45, 48), (48, 49)]


def _srow_of(core, pos, slot):
    """Table row for a node at (core, pos, slot) under chunk-major layout."""
    pos = np.asarray(pos)
    out = np.zeros(pos.shape, np.int64)
    base = 0
    for (lo, hi) in ACH:
        nrow = (hi - lo) * 128
        m = (pos >= lo) & (pos < hi)
        out[m] = base + core[m] * nrow + (pos[m] - lo) * 128 + slot[m]
        base += NCORES * nrow
    base = 0
    for (lo, hi) in BCH:
        nrow = (hi - lo) * 128
        m = (pos >= lo) & (pos < hi)
        out[m] = base + core[m] * nrow + (pos[m] - lo) * 128 + slot[m]
        base += NCORES * nrow
    return out


def _pack_nodes(src, dst):
    """Assign each node a (core, pos, slot); A-set = ids < A_SIZE -> pos < NPA.

    2D-banded packing: band by one in-degree coordinate, sort by the other
    within each band, so each 1024-node position block is homogeneous in
    (n_a, n_b) and per-block maxima (the slab padding) stay small."""
    deg = np.bincount(dst, minlength=N_NODES)
    n_a = np.bincount(dst[src < A_SIZE], minlength=N_NODES)
    n_b = deg - n_a

    ids = np.arange(N_NODES)
    a_ids = ids[:A_SIZE]
    b_ids = ids[A_SIZE:]

    def banded(idset, pri, sec, G):
        o = idset[np.argsort(-pri[idset], kind='stable')]
        out = []
        for i in range(0, len(o), 1024 * G):
            band = o[i:i + 1024 * G]
            out.append(band[np.argsort(-sec[band], kind='stable')])
        return np.concatenate(out)

    def chunk_cost(order, npos):
        tot, k = 0, 0
        for p in range(npos):
            take = order[k:k + 1024]
            k += len(take)
            tot += max(n_a[take].max(), 1) + max(n_b[take].max(), 1)
        return tot

    def best(idset, npos):
        cands = []
        for G in (2, 3, 4, 5, 6):
            for pri, sec in ((n_a, n_b), (n_b, n_a)):
                o = banded(idset, pri, sec, G)
                cands.append((chunk_cost(o, npos), o))
        cands.sort(key=lambda t: t[0])
        return cands[0][1]

    a_sorted = best(a_ids, NPA)
    b_sorted = best(b_ids, NPOS - NPA)

    core_of = np.full(N_NODES, -1, np.int32)
    pos_of = np.full(N_NODES, -1, np.int32)
    slot_of = np.full(N_NODES, -1, np.int32)

    def fill(sorted_ids, pos0):
        # snake deal: alternate core direction per position so the
        # degree-sorted blocks spread evenly across cores
        k = 0
        pos, ci = pos0, 0
        while k < len(sorted_ids):
            cap = 127 if pos in RES_POS else 128
            take = sorted_ids[k:k + cap]
            k += len(take)
            c = ci if (pos % 2 == 0) else NCORES - 1 - ci
            core_of[take] = c
            pos_of[take] = pos
            slot_of[take] = np.arange(len(take))
            ci += 1
            if ci == NCORES:
                ci = 0
                pos += 1

    fill(a_sorted, 0)
    fill(b_sorted, NPA)
    return core_of, pos_of, slot_of


def _build_grids(src, dst, core_of, pos_of, slot_of):
    """Per-core wrapped int16 idx arrays + per-position J_a/J_b schedules."""
    in_b = (pos_of[src] >= NPA).astype(np.int64)
    srow = _srow_of(core_of[src], pos_of[src], slot_of[src])
    dcore = core_of[dst]
    dpos = pos_of[dst]
    dslot = slot_of[dst]
    drow = dcore * (NPOS * 128) + dpos * 128 + dslot

    order = np.lexsort((srow, in_b, drow))
    gkey = drow[order] * 2 + in_b[order]
    newgrp = np.concatenate([[True], gkey[1:] != gkey[:-1]])
    gstart = np.maximum.accumulate(np.where(newgrp, np.arange(E_EDGES), 0))
    rank = np.arange(E_EDGES) - gstart
    j_in = np.empty(E_EDGES, np.int64)
    j_in[order] = rank

    na_e = np.where(in_b == 0, j_in + 1, 0)
    nb_e = np.where(in_b == 1, j_in + 1, 0)
    J_a = np.zeros(NPOS, np.int64)
    J_b = np.zeros(NPOS, np.int64)
    np.maximum.at(J_a, dpos, na_e)
    np.maximum.at(J_b, dpos, nb_e)
    J_a = np.maximum(J_a, 1)
    J_b = np.maximum(J_b, 1)

    grids = [[np.concatenate([
                np.full((J_a[p], 128), DUMMY_A_ROW, np.int64),
                np.full((J_b[p], 128), DUMMY_B_ROW, np.int64)])
              for p in range(NPOS)] for c in range(NCORES)]
    j_eff = np.where(in_b == 0, j_in, J_a[dpos] + j_in)
    for e in range(E_EDGES):
        grids[dcore[e]][dpos[e]][j_eff[e], dslot[e]] = srow[e]

    def wrap(grid):
        flat = grid.reshape(-1)
        w = flat.reshape(-1, 8, 16).transpose(2, 0, 1).reshape(16, -1)
        return np.tile(w, (8, 1)).astype(np.int16)

    idxT = []
    colbase = []
    for c in range(NCORES):
        parts = []
        cb = []
        col = 0
        for p in range(NPOS):
            a_w = wrap(grids[c][p][:J_a[p]])
            b_w = wrap(grids[c][p][J_a[p]:])
            cb.append((col, col + a_w.shape[1]))
            col += a_w.shape[1] + b_w.shape[1]
            parts.append(a_w)
            parts.append(b_w)
        idxT.append(np.concatenate(parts, axis=1))
        colbase = cb
    return idxT, J_a.tolist(), J_b.tolist(), colbase


def _fold_weights(W, al, ar):
    """[W | A | B] with A[k,h]=sum_d W[k,h*D+d]*al[h,d] (el), B likewise (er)."""
    H, D = al.shape
    Wr = W.reshape(W.shape[0], H, D)
    A = np.einsum('khd,hd->kh', Wr, al)
    B = np.einsum('khd,hd->kh', Wr, ar)
    return np.concatenate([W, A, B], axis=1).astype(np.float16)


def _build_program(J_a, J_b, colbase):
    import concourse.bacc as bacc
    import concourse.bass as bass
    import concourse.mybir as mybir
    from concourse.tile import TileContext

    f16 = mybir.dt.float16
    f32 = mybir.dt.float32
    AF = mybir.ActivationFunctionType
    OP = mybir.AluOpType

    TCOLS = [384, 384, 256]      # fp16 cols per table row per layer (stride)
    GCOLS = [256, 256, 160]      # ft cols per layer
    CCOLS = [264, 264, 168]      # node-phase out cols (ft + el4 + er4)
    TCC = [264, 264, 168]        # compact staged cols (ft + el4-as-f32)
    TOTC = colbase[-1][1] + J_b[-1] * 8
    Jt = [J_a[p] + J_b[p] for p in range(NPOS)]

    # g-tile size classes: top-2 positions -> 'gl' (bufs=2), rest 'gs'
    s_sorted = sorted(Jt, reverse=True)
    GL_MIN = s_sorted[2] + 1        # positions with Jt above this go to gl
    gl_bytes = 2 * s_sorted[0] * 384 * 2
    gs4_bytes = 4 * s_sorted[2] * 384 * 2
    gs_bufs = 4 if gl_bytes + gs4_bytes <= 135 * 1024 else 3

    nc = bacc.Bacc("TRN2", num_devices=NCORES, num_swdge_queues=4)
    featT = nc.dram_tensor("featT", [256, NPOS * 128], f16, kind="ExternalInput")
    idxT = nc.dram_tensor("idxT", [128, TOTC], mybir.dt.int16, kind="ExternalInput")
    Wes = [nc.dram_tensor(f"W{l}e", [256, CCOLS[l]], f16, kind="ExternalInput")
           for l in range(3)]
    out_d = nc.dram_tensor("out", [NPOS * 128, NCLS], f32, kind="ExternalOutput")

    agins = []   # per layer: list of (pos_lo, pos_hi, is_b, row_base, tensor)
    for l in range(3):
        chunks = []
        base = 0
        for (lo, hi) in ACH:
            t = nc.dram_tensor(f"aga{l}_{lo}", [(hi - lo) * 128, TCOLS[l]],
                               f16, kind="Internal")
            chunks.append((lo, hi, False, base, t))
            base += NCORES * (hi - lo) * 128
        base = 0
        for (lo, hi) in BCH:
            t = nc.dram_tensor(f"agb{l}_{lo}", [(hi - lo) * 128, TCOLS[l]],
                               f16, kind="Internal")
            chunks.append((lo, hi, True, base, t))
            base += NCORES * (hi - lo) * 128
        agins.append(chunks)
    table_a = [nc.dram_tensor(f"tablea{l}", [ROWS_A, TCOLS[l]], f16,
                              kind="Internal", addr_space="Shared")
               for l in range(3)]
    table_b = [nc.dram_tensor(f"tableb{l}", [ROWS_B, TCOLS[l]], f16,
                              kind="Internal", addr_space="Shared")
               for l in range(3)]

    qn = [0]

    def next_q():
        qn[0] = (qn[0] + 1) % 4
        return qn[0]

    with TileContext(nc) as tc:
        with tc.tile_pool(name="resident", bufs=1) as rp, \
             tc.tile_pool(name="work", bufs=3) as wp, \
             tc.tile_pool(name="gather", bufs=2) as gp, \
             tc.tile_pool(name="nps", bufs=3, space="PSUM") as nps, \
             tc.tile_pool(name="tps", bufs=4, space="PSUM") as tps:

            ia = rp.tile([128, TOTC], mybir.dt.int16)
            nc.sync.dma_start(ia[:], idxT[:])
            NP128 = NPOS * 128
            hTbig = rp.tile([128, 2 * NP128], f16)
            nc.sync.dma_start(hTbig[:, :NP128], featT[0:128, :])
            nc.sync.dma_start(hTbig[:, NP128:], featT[128:256, :])
            hT = [[bass.AP(hTbig.tensor,
                           hTbig[:].offset + k * NP128 + p * 128,
                           [hTbig[:].ap[0], [1, 128]])
                   for p in range(NPOS)] for k in range(2)]
            er_own = [rp.tile([128, 4], f32, tag=f"er{p}", name=f"er{p}")
                      for p in range(NPOS)]
            Wt = [rp.tile([128, 2, CCOLS[l]], f16, tag=f"Wt{l}", name=f"Wt{l}")
                  for l in range(3)]
            for l in range(3):
                nc.sync.dma_start(
                    Wt[l][:], Wes[l][:].rearrange("(k p) n -> p k n", k=2))
            # fp32 identity for PE transpose
            colv = rp.tile([128, 128], mybir.dt.int32)
            nc.gpsimd.iota(colv[:], [[1, 128]], base=0, channel_multiplier=0)
            rowv = rp.tile([128, 1], mybir.dt.int32)
            nc.gpsimd.iota(rowv[:], [[0, 1]], base=0, channel_multiplier=1)
            row_b = bass.AP(rowv.tensor, rowv[:].offset,
                            [rowv[:].ap[0], [0, 128]])
            identf = rp.tile([128, 128], f32)
            nc.vector.tensor_tensor(identf[:], colv[:], row_b, OP.is_equal)
            # -80 at partition 127, 0 elsewhere (dummy-row el marker)
            dmask = rp.tile([128, 1], f32)
            nc.vector.tensor_scalar(dmask[:], rowv[:], 127, -80.0,
                                    OP.is_equal, OP.mult)

            def node_phase(l, p):
                GC, CC = GCOLS[l], CCOLS[l]
                ps = nps.tile([128, CC], f32, tag="nodeps")
                for k in range(2):
                    nc.tensor.matmul(
                        ps[:], hT[k][p],
                        Wt[l][:].rearrange("p k n -> k p n")[k],
                        start=(k == 0), stop=(k == 1))
                nc.vector.tensor_copy(er_own[p][:], ps[:, GC + 4:GC + 8])
                stage = wp.tile([128, TCOLS[l]], f16, tag="stage")
                nc.vector.tensor_copy(stage[:, :GC], ps[:, :GC])
                st32 = stage[:].bitcast(f32)
                if p in RES_POS:
                    dm_b = bass.AP(dmask.tensor, dmask[:].offset,
                                   [dmask[:].ap[0], [0, 4]])
                    nc.vector.tensor_tensor(st32[:, GC // 2:GC // 2 + 4],
                                            ps[:, GC:GC + 4], dm_b, OP.add)
                else:
                    nc.vector.tensor_copy(st32[:, GC // 2:GC // 2 + 4],
                                          ps[:, GC:GC + 4])
                # stage -> the agin chunk containing position p (scalar-queue
                # HWDGE so the sync queue stays free)
                for (lo, hi, is_b, rb, t) in agins[l]:
                    if lo <= p < hi:
                        nc.scalar.dma_start(
                            t[(p - lo) * 128:(p - lo + 1) * 128, :], stage[:])
                # fire the collective for any chunk that just completed.
                # Fine-grained chunks keep each trigger's stage-DMA wait
                # short, so the in-order GpSimd queue is only briefly
                # blocked; the tiny last chunk keeps the AG that gates the
                # next sweep's gathers off the critical path.
                for (lo, hi, is_b, rb, t) in agins[l]:
                    if p == hi - 1:
                        tab = table_b[l] if is_b else table_a[l]
                        nrow = (hi - lo) * 128
                        nc.gpsimd.collective_compute(
                            "AllGather", OP.bypass,
                            replica_groups=[list(range(NCORES))],
                            ins=[t[:].opt()],
                            outs=[tab[rb:rb + NCORES * nrow, :].opt()])

            g_tiles = {}

            def edge_gather(l, p):
                GC, TC = GCOLS[l], TCOLS[l]
                Jl, Jh = J_a[p], J_b[p]
                Jtp = Jl + Jh
                locol, hicol = colbase[p]
                tag = "gl" if Jtp >= GL_MIN else "gs"
                nb = 2 if tag == "gl" else gs_bufs
                g = gp.tile([128, Jtp, 384], f16, tag=tag, bufs=nb, name="g")
                g_tiles[p] = g
                nc.gpsimd.dma_gather(
                    bass.AP(g.tensor, g[:].offset,
                            [g[:].ap[0], [TC, Jl], [1, TC]]),
                    table_a[l][:, :],
                    ia[:, locol:locol + Jl * 8], Jl * 128, Jl * 128, TC,
                    single_packet=False, queue_num=next_q())
                nc.gpsimd.dma_gather(
                    bass.AP(g.tensor, g[:].offset + Jl * TC,
                            [g[:].ap[0], [TC, Jh], [1, TC]]),
                    table_b[l][:, :],
                    ia[:, hicol:hicol + Jh * 8], Jh * 128, Jh * 128, TC,
                    single_packet=False, queue_num=next_q())

            def edge_phase(l, p):
                GC, TC = GCOLS[l], TCOLS[l]
                Jl, Jh = J_a[p], J_b[p]
                Jtp = Jl + Jh
                g = g_tiles.pop(p)
                # e = el + er_bcast   (h-major [128, 4, Jt] layout)
                elv = bass.AP(g.tensor, g[:].offset,
                              [g[:].ap[0], [TC, Jtp], [1, TC]]).bitcast(f32)
                el_hm = bass.AP(elv.tensor, elv.offset + GC // 2,
                                [elv.ap[0], [1, 4], [TC // 2, Jtp]])
                e_t = wp.tile([128, 4, Jtp], f32, tag="e")
                er_b = bass.AP(er_own[p].tensor, er_own[p][:].offset,
                               [er_own[p][:].ap[0], [1, 4], [0, Jtp]])
                nc.vector.tensor_tensor(e_t[:], el_hm, er_b, OP.add)
                # ex = exp(lrelu(e)) = max(exp(e), exp(0.2 e))
                ex1 = wp.tile([128, 4, Jtp], f32, tag="ex1")
                nc.scalar.activation(ex1[:], e_t[:], AF.Exp)
                ex2 = wp.tile([128, 4, Jtp], f32, tag="ex2")
                nc.scalar.activation(ex2[:], e_t[:], AF.Exp, scale=NEG_SLOPE)
                nc.vector.tensor_tensor(ex1[:], ex1[:], ex2[:], OP.max)
                # denom over j (inner-contiguous); rd = 1/max(denom, 1e-9)
                den = wp.tile([128, 4], f32, tag="den")
                nc.vector.tensor_reduce(den[:, :, None], ex1[:], op=OP.add,
                                        axis=mybir.AxisListType.X)
                nc.vector.tensor_scalar_max(den[:], den[:], 1e-9)
                rd = wp.tile([128, 4], f32, tag="rd")
                nc.vector.reciprocal(rd[:], den[:])
                # alpha = ex * rd_bcast  (fp16, h-major)
                alpha = wp.tile([128, 4, Jtp], f16, tag="alpha")
                rd_b = bass.AP(rd.tensor, rd[:].offset,
                               [rd[:].ap[0], [1, 4], [0, Jtp]])
                nc.vector.tensor_tensor(alpha[:], ex1[:], rd_b, OP.mult)
                # msg = alpha * ft, in place into g's ft cols
                D = GC // 4
                al_b = bass.AP(alpha.tensor, alpha[:].offset,
                               [alpha[:].ap[0], [1, Jtp], [Jtp, 4], [0, D]])
                ft4 = bass.AP(g.tensor, g[:].offset,
                              [g[:].ap[0], [TC, Jtp], [D, 4], [1, D]])
                nc.vector.tensor_tensor(ft4, ft4, al_b, OP.mult)
                # out_row = sum_j msg  (halving tree over j, in place)
                ro = wp.tile([128, GC], f32, tag="ro")

                def jsl(j0, cnt):
                    return bass.AP(g.tensor, g[:].offset + j0 * TC,
                                   [g[:].ap[0], [TC, cnt], [1, GC]])

                n = Jtp
                while n > 2:
                    h = n // 2
                    nc.vector.tensor_tensor(jsl(0, h), jsl(0, h),
                                            jsl(n - h, h), OP.add)
                    n -= h
                if n == 2:
                    nc.vector.tensor_tensor(ro[:], jsl(0, 1), jsl(1, 1),
                                            OP.add)
                else:
                    nc.vector.tensor_copy(ro[:], jsl(0, 1))
                if l < 2:
                    # h = elu(ro) = max(ro, min(exp(ro),1)-1); -> fp16 -> hT
                    ev = wp.tile([128, GC], f32, tag="ev")
                    nc.scalar.activation(ev[:], ro[:], AF.Exp)
                    nc.vector.tensor_scalar(ev[:], ev[:], 1.0, -1.0,
                                            OP.min, OP.add)
                    nc.vector.tensor_tensor(ro[:], ro[:], ev[:], OP.max)
                    for k in range(2):
                        tp = tps.tile([128, 128], f32, tag="trps")
                        nc.tensor.transpose(
                            tp[:], ro[:, k * 128:(k + 1) * 128], identf[:])
                        nc.vector.tensor_copy(hT[k][p], tp[:])
                else:
                    # logits = mean over heads; log_softmax
                    z = wp.tile([128, NCLS], f32, tag="z")
                    ro_h = bass.AP(ro.tensor, ro[:].offset,
                                   [ro[:].ap[0], [1, NCLS], [NCLS, 4]])
                    z_v = bass.AP(z.tensor, z[:].offset,
                                  [z[:].ap[0], [1, NCLS], [0, 1]])
                    nc.vector.tensor_reduce(z_v, ro_h, op=OP.add,
                                            axis=mybir.AxisListType.X)
                    nc.vector.tensor_scalar_mul(z[:], z[:], 0.25)
                    m = wp.tile([128, 1], f32, tag="m")
                    nc.vector.tensor_reduce(m[:], z[:], op=OP.max,
                                            axis=mybir.AxisListType.X)
                    nm = wp.tile([128, 1], f32, tag="nm")
                    nc.vector.tensor_scalar_mul(nm[:], m[:], -1.0)
                    ez = wp.tile([128, NCLS], f32, tag="ez")
                    s = wp.tile([128, 1], f32, tag="s")
                    nc.scalar.activation(ez[:], z[:], AF.Exp, bias=nm[:],
                                         accum_out=s[:])
                    lns = wp.tile([128, 1], f32, tag="lns")
                    nc.scalar.activation(lns[:], s[:], AF.Ln)
                    b = wp.tile([128, 1], f32, tag="b")
                    nc.vector.tensor_tensor(b[:], m[:], lns[:], OP.add)
                    lp = wp.tile([128, NCLS], f32, tag="lp")
                    b_b = bass.AP(b.tensor, b[:].offset,
                                  [b[:].ap[0], [0, NCLS]])
                    nc.vector.tensor_tensor(lp[:], z[:], b_b, OP.subtract)
                    nc.sync.dma_start(out_d[p * 128:(p + 1) * 128, :], lp[:])

            # sweep 0: node phase of layer 0 (+ its chunked all-gathers)
            for p in range(NPOS):
                node_phase(0, p)
            # sweeps 1,2: edge(l-1) interleaved with node(l)
            for l in (1, 2):
                for p in range(NPOS):
                    edge_gather(l - 1, p)
                for p in range(NPOS):
                    edge_phase(l - 1, p)
                    node_phase(l, p)
            # sweep 3: edge phase of layer 2 -> logits
            for p in range(NPOS):
                edge_gather(2, p)
            for p in range(NPOS):
                edge_phase(2, p)
    nc.compile()
    return nc


def _install_trace_shim():
    """Provide antenv.axon_hooks (get/set NTFF profile hook) when absent."""
    import sys, types
    try:
        from antenv.axon_hooks import get_axon_ntff_profile_hook  # noqa
        return
    except ImportError:
        pass
    mod = types.ModuleType("antenv.axon_hooks")
    _hook = [None]
    mod.set_axon_ntff_profile_hook = lambda h: _hook.__setitem__(0, h)
    mod.get_axon_ntff_profile_hook = lambda: _hook[0]
    sys.modules["antenv.axon_hooks"] = mod
    import antenv
    antenv.axon_hooks = mod
    if "/root/.axon_site" not in sys.path:
        sys.path.insert(0, "/root/.axon_site")
    from trn_agent_boot.trn_boot import _ntff_profile_via_ctypes
    mod.set_axon_ntff_profile_hook(
        _ntff_profile_via_ctypes("/opt/axon/libaxon_pjrt.so"))


def kernel(features, src, dst, W0, al0, ar0, W1, al1, ar1, W2, al2, ar2):
    import sys, os
    for pth in ("/root/axon_fix", "/opt/trn_rl_repo"):
        if os.path.isdir(pth) and pth not in sys.path:
            sys.path.insert(0, pth)
    if os.environ.get("KERNEL_TRACE"):
        _install_trace_shim()
    from concourse import bass_utils

    src = np.asarray(src).astype(np.int64)
    dst = np.asarray(dst).astype(np.int64)
    features = np.asarray(features, np.float32)

    core_of, pos_of, slot_of = _pack_nodes(src, dst)
    idxT, J_a, J_b, colbase = _build_grids(src, dst, core_of, pos_of, slot_of)
    Wes = [_fold_weights(np.asarray(W0, np.float32), np.asarray(al0), np.asarray(ar0)),
           _fold_weights(np.asarray(W1, np.float32), np.asarray(al1), np.asarray(ar1)),
           _fold_weights(np.asarray(W2, np.float32), np.asarray(al2), np.asarray(ar2))]

    # per-core featT [256, SLAB] fp16 in slot order
    featTs = []
    for c in range(NCORES):
        ft = np.zeros((256, NPOS * 128), np.float16)
        mask = core_of == c
        ids = np.arange(N_NODES)[mask]
        cols = pos_of[ids] * 128 + slot_of[ids]
        ft[:, cols] = features[ids].T.astype(np.float16)
        featTs.append(ft)

    nc = _build_program(J_a, J_b, colbase)
    ins = [{"featT": featTs[c], "idxT": idxT[c],
            "W0e": Wes[0], "W1e": Wes[1], "W2e": Wes[2]}
           for c in range(NCORES)]
    res = bass_utils.run_bass_kernel_spmd(
        nc, ins, core_ids=list(range(NCORES)),
        trace=bool(os.environ.get("KERNEL_TRACE")))
    if os.environ.get("KERNEL_TRACE"):
        print("HW exec time:", res.exec_time_ns, "ns")
        kernel.last_exec_ns = res.exec_time_ns
        kernel.last_trace = res.instructions_and_trace

    out = np.empty((N_NODES, NCLS), np.float32)
    for c in range(NCORES):
        mask = core_of == c
        ids = np.arange(N_NODES)[mask]
        rows = pos_of[ids] * 128 + slot_of[ids]
        out[ids] = res.results[c]["out"][rows]
    return out



# revision 12
# speedup vs baseline: 1.0760x; 1.0750x over previous
"""GAT 3-layer kernel for 8 TRN2 NeuronCores (slot-major edge-parallel design).

Sharding: dst nodes packed into 392 blocks of 128 slots (one slot = one SBUF
partition), blocks dealt to 8 cores x 49 positions. Edges live on the free dim
of each block (j-slabs), split lo/hi by source-table row (<32768 vs >=) so the
int16 dma_gather indices stay in range. Node features/tables are all-gathered
per layer (chunked, overlapped with the node phase); everything else is local.
"""
import numpy as np

N_NODES = 50000
E_EDGES = 800000
IN_FEATS = 256
HID = 64
HEADS = 4
NCLS = 40
NEG_SLOPE = 0.2

NCORES = 8
NPOS = 49                    # blocks per core
NPA = 32                     # positions in table A (rest in table B)
SLAB_A = NPA * 128           # 4096 rows/core in table A
SLAB_B = (NPOS - NPA) * 128  # 2176 rows/core in table B
ROWS_A = NCORES * SLAB_A     # 32768
ROWS_B = NCORES * SLAB_B     # 17408
A_SIZE = ROWS_A - 8          # A-region real-node capacity (8 reserved slots)
DUMMY_A_ROW = ROWS_A - 1     # (core 7, pos 31, slot 127)
DUMMY_B_ROW = ROWS_B - 1     # (core 7, pos 48, slot 127)
RES_POS = (NPA - 1, NPOS - 1)  # slot 127 reserved on every core

# AllGather chunk boundaries (position ranges); A chunks then B chunks.
# Tables are laid out chunk-major ([chunk][core][rows-in-chunk]) so each
# chunk's AllGather writes one contiguous range. Finer chunks let the
# collectives start while the sweep is still running; the last chunk of
# each region is a single position so the AG gating the next sweep's
# gathers is tiny.
ACH = [(0, 16), (16, 28), (28, 31), (31, 32)]
BCH = [(32, 40), (40, 45), (45, 48), (48, 49)]


def _srow_of(core, pos, slot):
    """Table row for a node at (core, pos, slot) under chunk-major layout."""
    pos = np.asarray(pos)
    out = np.zeros(pos.shape, np.int64)
    base = 0
    for (lo, hi) in ACH:
        nrow = (hi - lo) * 128
        m = (pos >= lo) & (pos < hi)
        out[m] = base + core[m] * nrow + (pos[m] - lo) * 128 + slot[m]
        base += NCORES * nrow
    base = 0
    for (lo, hi) in BCH:
        nrow = (hi - lo) * 128
        m = (pos >= lo) & (pos < hi)
        out[m] = base + core[m] * nrow + (pos[m] - lo) * 128 + slot[m]
        base += NCORES * nrow
    return out


def _pack_nodes(src, dst):
    """Assign each node a (core, pos, slot); A-set = ids < A_SIZE -> pos < NPA.

    2D-banded packing: band by one in-degree coordinate, sort by the other
    within each band, so each 1024-node position block is homogeneous in
    (n_a, n_b) and per-block maxima (the slab padding) stay small."""
    deg = np.bincount(dst, minlength=N_NODES)
    n_a = np.bincount(dst[src < A_SIZE], minlength=N_NODES)
    n_b = deg - n_a

    ids = np.arange(N_NODES)
    a_ids = ids[:A_SIZE]
    b_ids = ids[A_SIZE:]

    def banded(idset, pri, sec, G):
        o = idset[np.argsort(-pri[idset], kind='stable')]
        out = []
        for i in range(0, len(o), 1024 * G):
            band = o[i:i + 1024 * G]
            out.append(band[np.argsort(-sec[band], kind='stable')])
        return np.concatenate(out)

    def chunk_cost(order, npos):
        tot, k = 0, 0
        for p in range(npos):
            take = order[k:k + 1024]
            k += len(take)
            tot += max(n_a[take].max(), 1) + max(n_b[take].max(), 1)
        return tot

    def best(idset, npos):
        cands = []
        for G in (2, 3, 4, 5, 6):
            for pri, sec in ((n_a, n_b), (n_b, n_a)):
                o = banded(idset, pri, sec, G)
                cands.append((chunk_cost(o, npos), o))
        cands.sort(key=lambda t: t[0])
        return cands[0][1]

    a_sorted = best(a_ids, NPA)
    b_sorted = best(b_ids, NPOS - NPA)

    core_of = np.full(N_NODES, -1, np.int32)
    pos_of = np.full(N_NODES, -1, np.int32)
    slot_of = np.full(N_NODES, -1, np.int32)

    def fill(sorted_ids, pos0):
        # snake deal: alternate core direction per position so the
        # degree-sorted blocks spread evenly across cores
        k = 0
        pos, ci = pos0, 0
        while k < len(sorted_ids):
            cap = 127 if pos in RES_POS else 128
            take = sorted_ids[k:k + cap]
            k += len(take)
            c = ci if (pos % 2 == 0) else NCORES - 1 - ci
            core_of[take] = c
            pos_of[take] = pos
            slot_of[take] = np.arange(len(take))
            ci += 1
            if ci == NCORES:
                ci = 0
                pos += 1

    fill(a_sorted, 0)
    fill(b_sorted, NPA)
    return core_of, pos_of, slot_of


def _build_grids(src, dst, core_of, pos_of, slot_of):
    """Per-core wrapped int16 idx arrays + per-position J_a/J_b schedules."""
    in_b = (pos_of[src] >= NPA).astype(np.int64)
    srow = _srow_of(core_of[src], pos_of[src], slot_of[src])
    dcore = core_of[dst]
    dpos = pos_of[dst]
    dslot = slot_of[dst]
    drow = dcore * (NPOS * 128) + dpos * 128 + dslot

    order = np.lexsort((srow, in_b, drow))
    gkey = drow[order] * 2 + in_b[order]
    newgrp = np.concatenate([[True], gkey[1:] != gkey[:-1]])
    gstart = np.maximum.accumulate(np.where(newgrp, np.arange(E_EDGES), 0))
    rank = np.arange(E_EDGES) - gstart
    j_in = np.empty(E_EDGES, np.int64)
    j_in[order] = rank

    na_e = np.where(in_b == 0, j_in + 1, 0)
    nb_e = np.where(in_b == 1, j_in + 1, 0)
    J_a = np.zeros(NPOS, np.int64)
    J_b = np.zeros(NPOS, np.int64)
    np.maximum.at(J_a, dpos, na_e)
    np.maximum.at(J_b, dpos, nb_e)
    J_a = np.maximum(J_a, 1)
    J_b = np.maximum(J_b, 1)

    grids = [[np.concatenate([
                np.full((J_a[p], 128), DUMMY_A_ROW, np.int64),
                np.full((J_b[p], 128), DUMMY_B_ROW, np.int64)])
              for p in range(NPOS)] for c in range(NCORES)]
    j_eff = np.where(in_b == 0, j_in, J_a[dpos] + j_in)
    for e in range(E_EDGES):
        grids[dcore[e]][dpos[e]][j_eff[e], dslot[e]] = srow[e]

    def wrap(grid):
        flat = grid.reshape(-1)
        w = flat.reshape(-1, 8, 16).transpose(2, 0, 1).reshape(16, -1)
        return np.tile(w, (8, 1)).astype(np.int16)

    idxT = []
    colbase = []
    for c in range(NCORES):
        parts = []
        cb = []
        col = 0
        for p in range(NPOS):
            a_w = wrap(grids[c][p][:J_a[p]])
            b_w = wrap(grids[c][p][J_a[p]:])
            cb.append((col, col + a_w.shape[1]))
            col += a_w.shape[1] + b_w.shape[1]
            parts.append(a_w)
            parts.append(b_w)
        idxT.append(np.concatenate(parts, axis=1))
        colbase = cb
    return idxT, J_a.tolist(), J_b.tolist(), colbase


def _fold_weights(W, al, ar, ft_scale=1.0):
    """[W | A | B] with A[k,h]=sum_d W[k,h*D+d]*al[h,d] (el), B likewise (er).
    ft_scale scales only the W (message) part — used to fold the final
    mean-over-heads 1/H into layer 2's ft."""
    H, D = al.shape
    Wr = W.reshape(W.shape[0], H, D)
    A = np.einsum('khd,hd->kh', Wr, al)
    B = np.einsum('khd,hd->kh', Wr, ar)
    return np.concatenate([W * ft_scale, A, B], axis=1).astype(np.float16)


def _build_program(J_a, J_b, colbase):
    import concourse.bacc as bacc
    import concourse.bass as bass
    import concourse.mybir as mybir
    from concourse.tile import TileContext

    f16 = mybir.dt.float16
    f32 = mybir.dt.float32
    AF = mybir.ActivationFunctionType
    OP = mybir.AluOpType

    TCOLS = [384, 384, 256]      # fp16 cols per table row per layer (stride)
    GCOLS = [256, 256, 160]      # ft cols per layer
    CCOLS = [264, 264, 168]      # node-phase out cols (ft + el4 + er4)
    TCC = [264, 264, 168]        # compact staged cols (ft + el4-as-f32)
    TOTC = colbase[-1][1] + J_b[-1] * 8
    Jt = [J_a[p] + J_b[p] for p in range(NPOS)]

    # g-tile size classes: top-2 positions -> 'gl' (bufs=2), rest 'gs'
    s_sorted = sorted(Jt, reverse=True)
    GL_MIN = s_sorted[2] + 1        # positions with Jt above this go to gl
    gl_bytes = 2 * s_sorted[0] * 384 * 2
    gs4_bytes = 4 * s_sorted[2] * 384 * 2
    gs_bufs = 4 if gl_bytes + gs4_bytes <= 135 * 1024 else 3

    nc = bacc.Bacc("TRN2", num_devices=NCORES, num_swdge_queues=4)
    featT = nc.dram_tensor("featT", [256, NPOS * 128], f16, kind="ExternalInput")
    idxT = nc.dram_tensor("idxT", [128, TOTC], mybir.dt.int16, kind="ExternalInput")
    Wes = [nc.dram_tensor(f"W{l}e", [256, CCOLS[l]], f16, kind="ExternalInput")
           for l in range(3)]
    out_d = nc.dram_tensor("out", [NPOS * 128, NCLS], f32, kind="ExternalOutput")

    agins = []   # per layer: list of (pos_lo, pos_hi, is_b, row_base, tensor)
    for l in range(3):
        chunks = []
        base = 0
        for (lo, hi) in ACH:
            t = nc.dram_tensor(f"aga{l}_{lo}", [(hi - lo) * 128, TCOLS[l]],
                               f16, kind="Internal")
            chunks.append((lo, hi, False, base, t))
            base += NCORES * (hi - lo) * 128
        base = 0
        for (lo, hi) in BCH:
            t = nc.dram_tensor(f"agb{l}_{lo}", [(hi - lo) * 128, TCOLS[l]],
                               f16, kind="Internal")
            chunks.append((lo, hi, True, base, t))
            base += NCORES * (hi - lo) * 128
        agins.append(chunks)
    table_a = [nc.dram_tensor(f"tablea{l}", [ROWS_A, TCOLS[l]], f16,
                              kind="Internal", addr_space="Shared")
               for l in range(3)]
    table_b = [nc.dram_tensor(f"tableb{l}", [ROWS_B, TCOLS[l]], f16,
                              kind="Internal", addr_space="Shared")
               for l in range(3)]

    qn = [0]

    def next_q():
        qn[0] = (qn[0] + 1) % 4
        return qn[0]

    with TileContext(nc) as tc:
        with tc.tile_pool(name="resident", bufs=1) as rp, \
             tc.tile_pool(name="work", bufs=3) as wp, \
             tc.tile_pool(name="gather", bufs=2) as gp, \
             tc.tile_pool(name="nps", bufs=3, space="PSUM") as nps, \
             tc.tile_pool(name="tps", bufs=4, space="PSUM") as tps:

            ia = rp.tile([128, TOTC], mybir.dt.int16)
            nc.sync.dma_start(ia[:], idxT[:])
            NP128 = NPOS * 128
            hTbig = rp.tile([128, 2 * NP128], f16)
            nc.sync.dma_start(hTbig[:, :NP128], featT[0:128, :])
            nc.sync.dma_start(hTbig[:, NP128:], featT[128:256, :])
            hT = [[bass.AP(hTbig.tensor,
                           hTbig[:].offset + k * NP128 + p * 128,
                           [hTbig[:].ap[0], [1, 128]])
                   for p in range(NPOS)] for k in range(2)]
            er_own = [rp.tile([128, 4], f32, tag=f"er{p}", name=f"er{p}")
                      for p in range(NPOS)]
            Wt = [rp.tile([128, 2, CCOLS[l]], f16, tag=f"Wt{l}", name=f"Wt{l}")
                  for l in range(3)]
            for l in range(3):
                nc.sync.dma_start(
                    Wt[l][:], Wes[l][:].rearrange("(k p) n -> p k n", k=2))
            # fp32 identity for PE transpose
            colv = rp.tile([128, 128], mybir.dt.int32)
            nc.gpsimd.iota(colv[:], [[1, 128]], base=0, channel_multiplier=0)
            rowv = rp.tile([128, 1], mybir.dt.int32)
            nc.gpsimd.iota(rowv[:], [[0, 1]], base=0, channel_multiplier=1)
            row_b = bass.AP(rowv.tensor, rowv[:].offset,
                            [rowv[:].ap[0], [0, 128]])
            identf = rp.tile([128, 128], f32)
            nc.vector.tensor_tensor(identf[:], colv[:], row_b, OP.is_equal)
            # -80 at partition 127, 0 elsewhere (dummy-row el marker)
            dmask = rp.tile([128, 1], f32)
            nc.vector.tensor_scalar(dmask[:], rowv[:], 127, -80.0,
                                    OP.is_equal, OP.mult)
            # const tiles for contention-free tensor_tensor broadcasts
            ones_t = rp.tile([128, 1], f32)
            nc.vector.memset(ones_t[:], 1.0)
            epst = rp.tile([128, 1], f32)
            nc.vector.memset(epst[:], 1e-9)

            def node_phase(l, p):
                # copies/casts run on the scalar engine: DVE copy/cast ops
                # enter 2-port perf mode and fully block GpSimd SWDGE
                # descriptor generation (ACT never contends).
                GC, CC = GCOLS[l], CCOLS[l]
                ps = nps.tile([128, CC], f32, tag="nodeps")
                for k in range(2):
                    nc.tensor.matmul(
                        ps[:], hT[k][p],
                        Wt[l][:].rearrange("p k n -> k p n")[k],
                        start=(k == 0), stop=(k == 1))
                nc.scalar.activation(er_own[p][:], ps[:, GC + 4:GC + 8],
                                     AF.Copy)
                stage = wp.tile([128, TCOLS[l]], f16, tag="stage")
                nc.scalar.activation(stage[:, :GC], ps[:, :GC], AF.Copy)
                st32 = stage[:].bitcast(f32)
                if p in RES_POS:
                    dm_b = bass.AP(dmask.tensor, dmask[:].offset,
                                   [dmask[:].ap[0], [0, 4]])
                    nc.vector.tensor_tensor(st32[:, GC // 2:GC // 2 + 4],
                                            ps[:, GC:GC + 4], dm_b, OP.add)
                else:
                    nc.scalar.activation(st32[:, GC // 2:GC // 2 + 4],
                                         ps[:, GC:GC + 4], AF.Copy)
                # stage -> the agin chunk containing position p (scalar-queue
                # HWDGE so the sync queue stays free)
                for (lo, hi, is_b, rb, t) in agins[l]:
                    if lo <= p < hi:
                        nc.scalar.dma_start(
                            t[(p - lo) * 128:(p - lo + 1) * 128, :], stage[:])
                # fire the collective for any chunk that just completed.
                # Fine-grained chunks keep each trigger's stage-DMA wait
                # short, so the in-order GpSimd queue is only briefly
                # blocked; the tiny last chunk keeps the AG that gates the
                # next sweep's gathers off the critical path.
                for (lo, hi, is_b, rb, t) in agins[l]:
                    if p == hi - 1:
                        tab = table_b[l] if is_b else table_a[l]
                        nrow = (hi - lo) * 128
                        nc.gpsimd.collective_compute(
                            "AllGather", OP.bypass,
                            replica_groups=[list(range(NCORES))],
                            ins=[t[:].opt()],
                            outs=[tab[rb:rb + NCORES * nrow, :].opt()])

            g_tiles = {}

            def edge_gather(l, p):
                # split across all 4 SWDGE queues: gather throughput is
                # per-row per-queue (~122 rows/us/queue), so 4 concurrent
                # queues quadruple it
                GC, TC = GCOLS[l], TCOLS[l]
                Jl, Jh = J_a[p], J_b[p]
                Jtp = Jl + Jh
                locol, hicol = colbase[p]
                tag = "gl" if Jtp >= GL_MIN else "gs"
                nb = 2 if tag == "gl" else gs_bufs
                g = gp.tile([128, Jtp, 384], f16, tag=tag, bufs=nb, name="g")
                g_tiles[p] = g

                def gg(table, cb0, jd0, s0, nj):
                    nc.gpsimd.dma_gather(
                        bass.AP(g.tensor, g[:].offset + (jd0 + s0) * TC,
                                [g[:].ap[0], [TC, nj], [1, TC]]),
                        table[:, :],
                        ia[:, cb0 + s0 * 8:cb0 + (s0 + nj) * 8],
                        nj * 128, nj * 128, TC,
                        single_packet=False, queue_num=next_q())

                for (tab, cb0, jd0, J) in ((table_a[l], locol, 0, Jl),
                                           (table_b[l], hicol, Jl, Jh)):
                    if J >= 6:
                        h = J // 2
                        gg(tab, cb0, jd0, 0, h)
                        gg(tab, cb0, jd0, h, J - h)
                    else:
                        gg(tab, cb0, jd0, 0, J)

            def edge_phase(l, p):
                GC, TC = GCOLS[l], TCOLS[l]
                Jl, Jh = J_a[p], J_b[p]
                Jtp = Jl + Jh
                g = g_tiles.pop(p)
                # e = el + er_bcast   (h-major [128, 4, Jt] layout)
                elv = bass.AP(g.tensor, g[:].offset,
                              [g[:].ap[0], [TC, Jtp], [1, TC]]).bitcast(f32)
                el_hm = bass.AP(elv.tensor, elv.offset + GC // 2,
                                [elv.ap[0], [1, 4], [TC // 2, Jtp]])
                e_t = wp.tile([128, 4, Jtp], f32, tag="e")
                er_b = bass.AP(er_own[p].tensor, er_own[p][:].offset,
                               [er_own[p][:].ap[0], [1, 4], [0, Jtp]])
                nc.vector.tensor_tensor(e_t[:], el_hm, er_b, OP.add)
                # ex = exp(lrelu(e)) = max(exp(e), exp(0.2 e))
                ex1 = wp.tile([128, 4, Jtp], f32, tag="ex1")
                nc.scalar.activation(ex1[:], e_t[:], AF.Exp)
                ex2 = wp.tile([128, 4, Jtp], f32, tag="ex2")
                nc.scalar.activation(ex2[:], e_t[:], AF.Exp, scale=NEG_SLOPE)
                nc.vector.tensor_tensor(ex1[:], ex1[:], ex2[:], OP.max)
                # denom over j (inner-contiguous); rd = 1/max(denom, 1e-9)
                den = wp.tile([128, 4], f32, tag="den")
                nc.vector.tensor_reduce(den[:, :, None], ex1[:], op=OP.add,
                                        axis=mybir.AxisListType.X)
                eps_b = bass.AP(epst.tensor, epst[:].offset,
                                [epst[:].ap[0], [0, 4]])
                nc.vector.tensor_tensor(den[:], den[:], eps_b, OP.max)
                rd = wp.tile([128, 4], f32, tag="rd")
                nc.vector.reciprocal(rd[:], den[:])
                # alpha = ex * rd_bcast  (fp16, h-major)
                alpha = wp.tile([128, 4, Jtp], f16, tag="alpha")
                rd_b = bass.AP(rd.tensor, rd[:].offset,
                               [rd[:].ap[0], [1, 4], [0, Jtp]])
                nc.vector.tensor_tensor(alpha[:], ex1[:], rd_b, OP.mult)
                # msg = alpha * ft, in place into g's ft cols
                D = GC // 4
                al_b = bass.AP(alpha.tensor, alpha[:].offset,
                               [alpha[:].ap[0], [1, Jtp], [Jtp, 4], [0, D]])
                ft4 = bass.AP(g.tensor, g[:].offset,
                              [g[:].ap[0], [TC, Jtp], [D, 4], [1, D]])
                nc.vector.tensor_tensor(ft4, ft4, al_b, OP.mult)
                # out_row = sum_j msg  (halving tree over j, in place)
                ro = wp.tile([128, GC], f32, tag="ro")

                def jsl(j0, cnt):
                    return bass.AP(g.tensor, g[:].offset + j0 * TC,
                                   [g[:].ap[0], [TC, cnt], [1, GC]])

                n = Jtp
                while n > 2:
                    h = n // 2
                    nc.vector.tensor_tensor(jsl(0, h), jsl(0, h),
                                            jsl(n - h, h), OP.add)
                    n -= h
                if n == 2:
                    nc.vector.tensor_tensor(ro[:], jsl(0, 1), jsl(1, 1),
                                            OP.add)
                else:
                    nc.vector.tensor_copy(ro[:], jsl(0, 1))
                if l < 2:
                    # h = elu(ro) = max(ro, min(exp(ro),1)-1); -> fp16 -> hT
                    # (tensor_tensor only — DVE tensor_scalar would block
                    # GpSimd descriptor generation)
                    ev = wp.tile([128, GC], f32, tag="ev")
                    nc.scalar.activation(ev[:], ro[:], AF.Exp)
                    one_b = bass.AP(ones_t.tensor, ones_t[:].offset,
                                    [ones_t[:].ap[0], [0, GC]])
                    nc.vector.tensor_tensor(ev[:], ev[:], one_b, OP.min)
                    nc.vector.tensor_tensor(ev[:], ev[:], one_b, OP.subtract)
                    nc.vector.tensor_tensor(ro[:], ro[:], ev[:], OP.max)
                    for k in range(2):
                        tp = tps.tile([128, 128], f32, tag="trps")
                        nc.tensor.transpose(
                            tp[:], ro[:, k * 128:(k + 1) * 128], identf[:])
                        nc.scalar.activation(hT[k][p], tp[:], AF.Copy)
                else:
                    # logits = mean over heads (1/H folded into W2e ft);
                    # log_softmax
                    z = wp.tile([128, NCLS], f32, tag="z")
                    ro_h = bass.AP(ro.tensor, ro[:].offset,
                                   [ro[:].ap[0], [1, NCLS], [NCLS, 4]])
                    z_v = bass.AP(z.tensor, z[:].offset,
                                  [z[:].ap[0], [1, NCLS], [0, 1]])
                    nc.vector.tensor_reduce(z_v, ro_h, op=OP.add,
                                            axis=mybir.AxisListType.X)
                    m = wp.tile([128, 1], f32, tag="m")
                    nc.vector.tensor_reduce(m[:], z[:], op=OP.max,
                                            axis=mybir.AxisListType.X)
                    nm = wp.tile([128, 1], f32, tag="nm")
                    nc.scalar.activation(nm[:], m[:], AF.Copy, scale=-1.0)
                    ez = wp.tile([128, NCLS], f32, tag="ez")
                    s = wp.tile([128, 1], f32, tag="s")
                    nc.scalar.activation(ez[:], z[:], AF.Exp, bias=nm[:],
                                         accum_out=s[:])
                    lns = wp.tile([128, 1], f32, tag="lns")
                    nc.scalar.activation(lns[:], s[:], AF.Ln)
                    b = wp.tile([128, 1], f32, tag="b")
                    nc.vector.tensor_tensor(b[:], m[:], lns[:], OP.add)
                    lp = wp.tile([128, NCLS], f32, tag="lp")
                    b_b = bass.AP(b.tensor, b[:].offset,
                                  [b[:].ap[0], [0, NCLS]])
                    nc.vector.tensor_tensor(lp[:], z[:], b_b, OP.subtract)
                    nc.sync.dma_start(out_d[p * 128:(p + 1) * 128, :], lp[:])

            # sweep 0: node phase of layer 0 (+ its chunked all-gathers)
            for p in range(NPOS):
                node_phase(0, p)
            # sweeps 1,2: edge(l-1) interleaved with node(l)
            for l in (1, 2):
                for p in range(NPOS):
                    edge_gather(l - 1, p)
                for p in range(NPOS):
                    edge_phase(l - 1, p)
                    node_phase(l, p)
            # sweep 3: edge phase of layer 2 -> logits
            for p in range(NPOS):
                edge_gather(2, p)
            for p in range(NPOS):
                edge_phase(2, p)
    nc.compile()
    return nc


def _install_trace_shim():
    """Provide antenv.axon_hooks (get/set NTFF profile hook) when absent."""
    import sys, types
    try:
        from antenv.axon_hooks import get_axon_ntff_profile_hook  # noqa
        return
    except ImportError:
        pass
    mod = types.ModuleType("antenv.axon_hooks")
    _hook = [None]
    mod.set_axon_ntff_profile_hook = lambda h: _hook.__setitem__(0, h)
    mod.get_axon_ntff_profile_hook = lambda: _hook[0]
    sys.modules["antenv.axon_hooks"] = mod
    import antenv
    antenv.axon_hooks = mod
    if "/root/.axon_site" not in sys.path:
        sys.path.insert(0, "/root/.axon_site")
    from trn_agent_boot.trn_boot import _ntff_profile_via_ctypes
    mod.set_axon_ntff_profile_hook(
        _ntff_profile_via_ctypes("/opt/axon/libaxon_pjrt.so"))


def kernel(features, src, dst, W0, al0, ar0, W1, al1, ar1, W2, al2, ar2):
    import sys, os
    for pth in ("/root/axon_fix", "/opt/trn_rl_repo"):
        if os.path.isdir(pth) and pth not in sys.path:
            sys.path.insert(0, pth)
    if os.environ.get("KERNEL_TRACE"):
        _install_trace_shim()
    from concourse import bass_utils

    src = np.asarray(src).astype(np.int64)
    dst = np.asarray(dst).astype(np.int64)
    features = np.asarray(features, np.float32)

    core_of, pos_of, slot_of = _pack_nodes(src, dst)
    idxT, J_a, J_b, colbase = _build_grids(src, dst, core_of, pos_of, slot_of)
    Wes = [_fold_weights(np.asarray(W0, np.float32), np.asarray(al0), np.asarray(ar0)),
           _fold_weights(np.asarray(W1, np.float32), np.asarray(al1), np.asarray(ar1)),
           _fold_weights(np.asarray(W2, np.float32), np.asarray(al2), np.asarray(ar2),
                         ft_scale=1.0 / HEADS)]

    # per-core featT [256, SLAB] fp16 in slot order
    featTs = []
    for c in range(NCORES):
        ft = np.zeros((256, NPOS * 128), np.float16)
        mask = core_of == c
        ids = np.arange(N_NODES)[mask]
        cols = pos_of[ids] * 128 + slot_of[ids]
        ft[:, cols] = features[ids].T.astype(np.float16)
        featTs.append(ft)

    nc = _build_program(J_a, J_b, colbase)
    ins = [{"featT": featTs[c], "idxT": idxT[c],
            "W0e": Wes[0], "W1e": Wes[1], "W2e": Wes[2]}
           for c in range(NCORES)]
    res = bass_utils.run_bass_kernel_spmd(
        nc, ins, core_ids=list(range(NCORES)),
        trace=bool(os.environ.get("KERNEL_TRACE")))
    if os.environ.get("KERNEL_TRACE"):
        print("HW exec time:", res.exec_time_ns, "ns")
        kernel.last_exec_ns = res.exec_time_ns
        kernel.last_trace = res.instructions_and_trace

    out = np.empty((N_NODES, NCLS), np.float32)
    for c in range(NCORES):
        mask = core_of == c
        ids = np.arange(N_NODES)[mask]
        rows = pos_of[ids] * 128 + slot_of[ids]
        out[ids] = res.results[c]["out"][rows]
    return out



# revision 21
# speedup vs baseline: 1.1891x; 1.1051x over previous
"""GAT 3-layer kernel for 8 TRN2 NeuronCores (slot-major edge-parallel design).

Sharding: dst nodes packed into 392 blocks of 128 slots (one slot = one SBUF
partition), blocks dealt to 8 cores x 49 positions. Edges live on the free dim
of each block (j-slabs), split lo/hi by source-table row (<32768 vs >=) so the
int16 dma_gather indices stay in range. Node features/tables are all-gathered
per layer (chunked, overlapped with the node phase); everything else is local.
"""
import numpy as np

N_NODES = 50000
E_EDGES = 800000
IN_FEATS = 256
HID = 64
HEADS = 4
NCLS = 40
NEG_SLOPE = 0.2

NCORES = 8
NPOS = 49                    # blocks per core
NPA = 32                     # positions in table A (rest in table B)
SLAB_A = NPA * 128           # 4096 rows/core in table A
SLAB_B = (NPOS - NPA) * 128  # 2176 rows/core in table B
ROWS_A = NCORES * SLAB_A     # 32768
ROWS_B = NCORES * SLAB_B     # 17408
A_SIZE = ROWS_A - 8          # A-region real-node capacity (8 reserved slots)
DUMMY_A_ROW = ROWS_A - 1     # (core 7, pos 31, slot 127)
DUMMY_B_ROW = ROWS_B - 1     # (core 7, pos 48, slot 127)
RES_POS = (NPA - 1, NPOS - 1)  # slot 127 reserved on every core

def _srow_of(core, pos, slot, ach, bch):
    """Table row for a node at (core, pos, slot) under chunk-major layout."""
    pos = np.asarray(pos)
    out = np.zeros(pos.shape, np.int64)
    base = 0
    for (lo, hi) in ach:
        nrow = (hi - lo) * 128
        m = (pos >= lo) & (pos < hi)
        out[m] = base + core[m] * nrow + (pos[m] - lo) * 128 + slot[m]
        base += NCORES * nrow
    base = 0
    for (lo, hi) in bch:
        nrow = (hi - lo) * 128
        m = (pos >= lo) & (pos < hi)
        out[m] = base + core[m] * nrow + (pos[m] - lo) * 128 + slot[m]
        base += NCORES * nrow
    return out


def _reorder_positions(src, dst, core_of, pos_of, slot_of):
    """Relabel positions so process order (=label order per region) is a
    light-heavy-light pyramid: early labels are light (their AG chunks fire
    early in the sweep), heavies sit mid-sweep, and the final label of each
    region is light so the tail AllGather gating the next sweep is tiny.
    Returns (new pos_of, ACH, BCH, ORDER)."""
    in_b_node = pos_of >= NPA
    na = np.zeros(NPOS, np.int64)
    nb = np.zeros(NPOS, np.int64)
    # per-(core,pos,slot) counts of A/B edges -> per-pos max (the J profile)
    eb = (pos_of[src] >= NPA).astype(np.int64)
    key = (core_of[dst] * NPOS + pos_of[dst]) * 128 + slot_of[dst]
    for v, arr in ((0, na), (1, nb)):
        m = eb == v
        cnt = np.bincount(key[m], minlength=NCORES * NPOS * 128)
        mx = cnt.reshape(NCORES, NPOS, 128).max(axis=(0, 2))
        arr[:] = mx
    jt = np.maximum(na, 1) + np.maximum(nb, 1)

    def pyramid(labels):
        asc = sorted(labels, key=lambda p: jt[p])
        return asc[0::2] + asc[1::2][::-1]

    seqA = pyramid(list(range(NPA)))          # old labels, process order
    seqB = pyramid(list(range(NPA, NPOS)))
    # relabel: process order becomes label order within each region
    newpos = np.empty(NPOS, np.int64)
    for i, op in enumerate(seqA):
        newpos[op] = i
    for i, op in enumerate(seqB):
        newpos[op] = NPA + i
    pos_of2 = newpos[pos_of]
    jtA = jt[seqA]
    jtB = jt[seqB]

    def chunks(jts, lo0, fracs):
        cum = np.cumsum(jts) / jts.sum()
        n = len(jts)
        bnds = [lo0]
        for f in fracs:
            b = lo0 + int(np.searchsorted(cum, f) + 1)
            if b > bnds[-1] and b < lo0 + n - 1:
                bnds.append(b)
        if bnds[-1] != lo0 + n - 1:
            bnds.append(lo0 + n - 1)      # last chunk = single light position
        bnds.append(lo0 + n)
        return [(bnds[i], bnds[i + 1]) for i in range(len(bnds) - 1)]

    ach = chunks(jtA, 0, [0.33, 0.62, 0.86])
    bch = chunks(jtB, NPA, [0.5, 0.84])
    # merged process order: interleave A and B labels by cumulative fraction
    cumA = np.cumsum(jtA) / jtA.sum()
    cumB = np.cumsum(jtB) / jtB.sum()
    tagged = [(cumA[i], i) for i in range(NPA)] + \
             [(cumB[i], NPA + i) for i in range(NPOS - NPA)]
    order = [p for _, p in sorted(tagged)]
    return pos_of2, ach, bch, order


def _pack_nodes(src, dst):
    """Assign each node a (core, pos, slot); A-set = ids < A_SIZE -> pos < NPA.

    2D-banded packing: band by one in-degree coordinate, sort by the other
    within each band, so each 1024-node position block is homogeneous in
    (n_a, n_b) and per-block maxima (the slab padding) stay small."""
    deg = np.bincount(dst, minlength=N_NODES)
    n_a = np.bincount(dst[src < A_SIZE], minlength=N_NODES)
    n_b = deg - n_a

    ids = np.arange(N_NODES)
    a_ids = ids[:A_SIZE]
    b_ids = ids[A_SIZE:]

    def banded(idset, pri, sec, G):
        o = idset[np.argsort(-pri[idset], kind='stable')]
        out = []
        for i in range(0, len(o), 1024 * G):
            band = o[i:i + 1024 * G]
            out.append(band[np.argsort(-sec[band], kind='stable')])
        return np.concatenate(out)

    def chunk_cost(order, npos):
        tot, k = 0, 0
        for p in range(npos):
            take = order[k:k + 1024]
            k += len(take)
            tot += max(n_a[take].max(), 1) + max(n_b[take].max(), 1)
        return tot

    def best(idset, npos):
        cands = []
        for G in (2, 3, 4, 5, 6):
            for pri, sec in ((n_a, n_b), (n_b, n_a)):
                o = banded(idset, pri, sec, G)
                cands.append((chunk_cost(o, npos), o))
        cands.sort(key=lambda t: t[0])
        return cands[0][1]

    a_sorted = best(a_ids, NPA)
    b_sorted = best(b_ids, NPOS - NPA)

    core_of = np.full(N_NODES, -1, np.int32)
    pos_of = np.full(N_NODES, -1, np.int32)
    slot_of = np.full(N_NODES, -1, np.int32)

    def fill(sorted_ids, pos0):
        # snake deal: alternate core direction per position so the
        # degree-sorted blocks spread evenly across cores
        k = 0
        pos, ci = pos0, 0
        while k < len(sorted_ids):
            take = sorted_ids[k:k + 128]
            k += len(take)
            c = ci if (pos % 2 == 0) else NCORES - 1 - ci
            core_of[take] = c
            pos_of[take] = pos
            slot_of[take] = np.arange(len(take))
            ci += 1
            if ci == NCORES:
                ci = 0
                pos += 1

    fill(a_sorted, 0)
    fill(b_sorted, NPA)
    return core_of, pos_of, slot_of


def _evict_reserved(core_of, pos_of, slot_of):
    """After relabeling, slot 127 of the RES_POS positions (labels NPA-1 and
    NPOS-1, every core) must be node-free: the node phase stamps the dummy
    el marker (-80) on partition 127 there. Move any occupants to free
    slots elsewhere in their region."""
    occ = np.zeros((NCORES, NPOS, 128), bool)
    occ[core_of, pos_of, slot_of] = True
    for rp, lo, hi in ((NPA - 1, 0, NPA), (NPOS - 1, NPA, NPOS)):
        offenders = np.where((pos_of == rp) & (slot_of == 127))[0]
        free = np.argwhere(~occ[:, lo:hi, :])
        free = [(c, lo + p, s) for c, p, s in free
                if not (p + lo == rp and s == 127)]
        assert len(free) >= len(offenders)
        for i, node in enumerate(offenders):
            c, p, s = free[i]
            occ[core_of[node], pos_of[node], slot_of[node]] = False
            core_of[node], pos_of[node], slot_of[node] = c, p, s
            occ[c, p, s] = True


def _build_grids(src, dst, core_of, pos_of, slot_of, ach, bch):
    """Per-core wrapped int16 idx arrays + per-position J_a/J_b schedules."""
    in_b = (pos_of[src] >= NPA).astype(np.int64)
    srow = _srow_of(core_of[src], pos_of[src], slot_of[src], ach, bch)
    dcore = core_of[dst]
    dpos = pos_of[dst]
    dslot = slot_of[dst]
    drow = dcore * (NPOS * 128) + dpos * 128 + dslot

    order = np.lexsort((srow, in_b, drow))
    gkey = drow[order] * 2 + in_b[order]
    newgrp = np.concatenate([[True], gkey[1:] != gkey[:-1]])
    gstart = np.maximum.accumulate(np.where(newgrp, np.arange(E_EDGES), 0))
    rank = np.arange(E_EDGES) - gstart
    j_in = np.empty(E_EDGES, np.int64)
    j_in[order] = rank

    na_e = np.where(in_b == 0, j_in + 1, 0)
    nb_e = np.where(in_b == 1, j_in + 1, 0)
    J_a = np.zeros(NPOS, np.int64)
    J_b = np.zeros(NPOS, np.int64)
    np.maximum.at(J_a, dpos, na_e)
    np.maximum.at(J_b, dpos, nb_e)
    J_a = np.maximum(J_a, 1)
    J_b = np.maximum(J_b, 1)

    grids = [[np.concatenate([
                np.full((J_a[p], 128), DUMMY_A_ROW, np.int64),
                np.full((J_b[p], 128), DUMMY_B_ROW, np.int64)])
              for p in range(NPOS)] for c in range(NCORES)]
    j_eff = np.where(in_b == 0, j_in, J_a[dpos] + j_in)
    for e in range(E_EDGES):
        grids[dcore[e]][dpos[e]][j_eff[e], dslot[e]] = srow[e]

    def wrap(grid):
        flat = grid.reshape(-1)
        w = flat.reshape(-1, 8, 16).transpose(2, 0, 1).reshape(16, -1)
        return np.tile(w, (8, 1)).astype(np.int16)

    idxT = []
    colbase = []
    for c in range(NCORES):
        parts = []
        cb = []
        col = 0
        for p in range(NPOS):
            a_w = wrap(grids[c][p][:J_a[p]])
            b_w = wrap(grids[c][p][J_a[p]:])
            cb.append((col, col + a_w.shape[1]))
            col += a_w.shape[1] + b_w.shape[1]
            parts.append(a_w)
            parts.append(b_w)
        idxT.append(np.concatenate(parts, axis=1))
        colbase = cb
    return idxT, J_a.tolist(), J_b.tolist(), colbase


def _fold_weights(W, al, ar, ft_scale=1.0):
    """[W | A | B] with A[k,h]=sum_d W[k,h*D+d]*al[h,d] (el), B likewise (er).
    ft_scale scales only the W (message) part — used to fold the final
    mean-over-heads 1/H into layer 2's ft."""
    H, D = al.shape
    Wr = W.reshape(W.shape[0], H, D)
    A = np.einsum('khd,hd->kh', Wr, al)
    B = np.einsum('khd,hd->kh', Wr, ar)
    return np.concatenate([W * ft_scale, A, B], axis=1).astype(np.float16)


def _build_program(J_a, J_b, colbase, ACH, BCH, ORDER):
    import concourse.bacc as bacc
    import concourse.bass as bass
    import concourse.mybir as mybir
    from concourse.tile import TileContext

    f16 = mybir.dt.float16
    f32 = mybir.dt.float32
    AF = mybir.ActivationFunctionType
    OP = mybir.AluOpType

    TCOLS = [384, 384, 256]      # fp16 cols per table row per layer (stride)
    GCOLS = [256, 256, 160]      # ft cols per layer
    CCOLS = [264, 264, 168]      # node-phase out cols (ft + el4 + er4)
    TCC = [264, 264, 168]        # compact staged cols (ft + el4-as-f32)
    TOTC = colbase[-1][1] + J_b[-1] * 8
    Jt = [J_a[p] + J_b[p] for p in range(NPOS)]

    # g-tile size classes: top-2 positions -> 'gl' (bufs=1; the reorder
    # spaces them far apart in process order so a single buffer never
    # stalls), rest 'gs' with a deep ring to keep the gather queues fed
    s_sorted = sorted(Jt, reverse=True)
    GL_MIN = s_sorted[2] + 1        # positions with Jt above this go to gl
    gl_bufs = 1
    gl_bytes = gl_bufs * s_sorted[0] * 384 * 2
    gs4_bytes = 4 * s_sorted[2] * 384 * 2
    gs_bufs = 4 if gl_bytes + gs4_bytes <= 138 * 1024 else 3

    nc = bacc.Bacc("TRN2", num_devices=NCORES, num_swdge_queues=4)
    featT = nc.dram_tensor("featT", [256, NPOS * 128], f16, kind="ExternalInput")
    idxT = nc.dram_tensor("idxT", [128, TOTC], mybir.dt.int16, kind="ExternalInput")
    Wes = [nc.dram_tensor(f"W{l}e", [256, CCOLS[l]], f16, kind="ExternalInput")
           for l in range(3)]
    out_d = nc.dram_tensor("out", [NPOS * 128, NCLS], f32, kind="ExternalOutput")

    agins = []   # per layer: list of (pos_lo, pos_hi, is_b, row_base, tensor)
    for l in range(3):
        chunks = []
        base = 0
        for (lo, hi) in ACH:
            t = nc.dram_tensor(f"aga{l}_{lo}", [(hi - lo) * 128, TCOLS[l]],
                               f16, kind="Internal")
            chunks.append((lo, hi, False, base, t))
            base += NCORES * (hi - lo) * 128
        base = 0
        for (lo, hi) in BCH:
            t = nc.dram_tensor(f"agb{l}_{lo}", [(hi - lo) * 128, TCOLS[l]],
                               f16, kind="Internal")
            chunks.append((lo, hi, True, base, t))
            base += NCORES * (hi - lo) * 128
        agins.append(chunks)
    table_a = [nc.dram_tensor(f"tablea{l}", [ROWS_A, TCOLS[l]], f16,
                              kind="Internal", addr_space="Shared")
               for l in range(3)]
    table_b = [nc.dram_tensor(f"tableb{l}", [ROWS_B, TCOLS[l]], f16,
                              kind="Internal", addr_space="Shared")
               for l in range(3)]

    qn = [0]

    def next_q():
        qn[0] = (qn[0] + 1) % 4
        return qn[0]

    with TileContext(nc) as tc:
        with tc.tile_pool(name="resident", bufs=1) as rp, \
             tc.tile_pool(name="work", bufs=3) as wp, \
             tc.tile_pool(name="gather", bufs=2) as gp, \
             tc.tile_pool(name="nps", bufs=3, space="PSUM") as nps, \
             tc.tile_pool(name="tps", bufs=4, space="PSUM") as tps:

            ia = rp.tile([128, TOTC], mybir.dt.int16)
            nc.sync.dma_start(ia[:], idxT[:])
            NP128 = NPOS * 128
            hTbig = rp.tile([128, 2 * NP128], f16)
            nc.sync.dma_start(hTbig[:, :NP128], featT[0:128, :])
            nc.sync.dma_start(hTbig[:, NP128:], featT[128:256, :])
            hT = [[bass.AP(hTbig.tensor,
                           hTbig[:].offset + k * NP128 + p * 128,
                           [hTbig[:].ap[0], [1, 128]])
                   for p in range(NPOS)] for k in range(2)]
            er_own = [rp.tile([128, 4], f32, tag=f"er{p}", name=f"er{p}")
                      for p in range(NPOS)]
            Wt = [rp.tile([128, 2, CCOLS[l]], f16, tag=f"Wt{l}", name=f"Wt{l}")
                  for l in range(3)]
            for l in range(3):
                nc.sync.dma_start(
                    Wt[l][:], Wes[l][:].rearrange("(k p) n -> p k n", k=2))
            # fp32 identity for PE transpose
            colv = rp.tile([128, 128], mybir.dt.int32)
            nc.gpsimd.iota(colv[:], [[1, 128]], base=0, channel_multiplier=0)
            rowv = rp.tile([128, 1], mybir.dt.int32)
            nc.gpsimd.iota(rowv[:], [[0, 1]], base=0, channel_multiplier=1)
            row_b = bass.AP(rowv.tensor, rowv[:].offset,
                            [rowv[:].ap[0], [0, 128]])
            identf = rp.tile([128, 128], f32)
            nc.vector.tensor_tensor(identf[:], colv[:], row_b, OP.is_equal)
            # -80 at partition 127, 0 elsewhere (dummy-row el marker)
            dmask = rp.tile([128, 1], f32)
            nc.vector.tensor_scalar(dmask[:], rowv[:], 127, -80.0,
                                    OP.is_equal, OP.mult)
            # const tiles for contention-free tensor_tensor broadcasts
            ones_t = rp.tile([128, 1], f32)
            nc.vector.memset(ones_t[:], 1.0)
            epst = rp.tile([128, 1], f32)
            nc.vector.memset(epst[:], 1e-9)

            def node_phase(l, p):
                # copies/casts run on the scalar engine: DVE copy/cast ops
                # enter 2-port perf mode and fully block GpSimd SWDGE
                # descriptor generation (ACT never contends).
                GC, CC = GCOLS[l], CCOLS[l]
                ps = nps.tile([128, CC], f32, tag="nodeps")
                for k in range(2):
                    nc.tensor.matmul(
                        ps[:], hT[k][p],
                        Wt[l][:].rearrange("p k n -> k p n")[k],
                        start=(k == 0), stop=(k == 1))
                nc.scalar.activation(er_own[p][:], ps[:, GC + 4:GC + 8],
                                     AF.Copy)
                stage = wp.tile([128, TCOLS[l]], f16, tag="stage")
                nc.scalar.activation(stage[:, :GC], ps[:, :GC], AF.Copy)
                st32 = stage[:].bitcast(f32)
                if p in RES_POS:
                    dm_b = bass.AP(dmask.tensor, dmask[:].offset,
                                   [dmask[:].ap[0], [0, 4]])
                    nc.vector.tensor_tensor(st32[:, GC // 2:GC // 2 + 4],
                                            ps[:, GC:GC + 4], dm_b, OP.add)
                else:
                    nc.scalar.activation(st32[:, GC // 2:GC // 2 + 4],
                                         ps[:, GC:GC + 4], AF.Copy)
                # stage -> the agin chunk containing position p (scalar-queue
                # HWDGE so the sync queue stays free)
                for (lo, hi, is_b, rb, t) in agins[l]:
                    if lo <= p < hi:
                        nc.scalar.dma_start(
                            t[(p - lo) * 128:(p - lo + 1) * 128, :], stage[:])
                # fire the collective for any chunk that just completed.
                # Fine-grained chunks keep each trigger's stage-DMA wait
                # short, so the in-order GpSimd queue is only briefly
                # blocked; the tiny last chunk keeps the AG that gates the
                # next sweep's gathers off the critical path.
                for (lo, hi, is_b, rb, t) in agins[l]:
                    if p == hi - 1:
                        tab = table_b[l] if is_b else table_a[l]
                        nrow = (hi - lo) * 128
                        nc.gpsimd.collective_compute(
                            "AllGather", OP.bypass,
                            replica_groups=[list(range(NCORES))],
                            ins=[t[:].opt()],
                            outs=[tab[rb:rb + NCORES * nrow, :].opt()])

            g_tiles = {}

            def edge_gather(l, p):
                # split across all 4 SWDGE queues: gather throughput is
                # per-row per-queue (~122 rows/us/queue), so 4 concurrent
                # queues quadruple it
                GC, TC = GCOLS[l], TCOLS[l]
                Jl, Jh = J_a[p], J_b[p]
                Jtp = Jl + Jh
                locol, hicol = colbase[p]
                tag = "gl" if Jtp >= GL_MIN else "gs"
                nb = gl_bufs if tag == "gl" else gs_bufs
                g = gp.tile([128, Jtp, 384], f16, tag=tag, bufs=nb, name="g")
                g_tiles[p] = g

                def gg(table, cb0, jd0, s0, nj):
                    nc.gpsimd.dma_gather(
                        bass.AP(g.tensor, g[:].offset + (jd0 + s0) * TC,
                                [g[:].ap[0], [TC, nj], [1, TC]]),
                        table[:, :],
                        ia[:, cb0 + s0 * 8:cb0 + (s0 + nj) * 8],
                        nj * 128, nj * 128, TC,
                        single_packet=False, queue_num=next_q())

                for (tab, cb0, jd0, J) in ((table_a[l], locol, 0, Jl),
                                           (table_b[l], hicol, Jl, Jh)):
                    if J >= 6:
                        h = J // 2
                        gg(tab, cb0, jd0, 0, h)
                        gg(tab, cb0, jd0, h, J - h)
                    else:
                        gg(tab, cb0, jd0, 0, J)

            def edge_phase(l, p):
                GC, TC = GCOLS[l], TCOLS[l]
                Jl, Jh = J_a[p], J_b[p]
                Jtp = Jl + Jh
                g = g_tiles.pop(p)
                # e = el + er_bcast   (h-major [128, 4, Jt] layout)
                elv = bass.AP(g.tensor, g[:].offset,
                              [g[:].ap[0], [TC, Jtp], [1, TC]]).bitcast(f32)
                el_hm = bass.AP(elv.tensor, elv.offset + GC // 2,
                                [elv.ap[0], [1, 4], [TC // 2, Jtp]])
                e_t = wp.tile([128, 4, Jtp], f32, tag="e")
                er_b = bass.AP(er_own[p].tensor, er_own[p][:].offset,
                               [er_own[p][:].ap[0], [1, 4], [0, Jtp]])
                nc.vector.tensor_tensor(e_t[:], el_hm, er_b, OP.add)
                # ex = exp(lrelu(e)) = max(exp(e), exp(0.2 e))
                ex1 = wp.tile([128, 4, Jtp], f32, tag="ex1")
                nc.scalar.activation(ex1[:], e_t[:], AF.Exp)
                ex2 = wp.tile([128, 4, Jtp], f32, tag="ex2")
                nc.scalar.activation(ex2[:], e_t[:], AF.Exp, scale=NEG_SLOPE)
                nc.vector.tensor_tensor(ex1[:], ex1[:], ex2[:], OP.max)
                # denom over j (inner-contiguous); rd = 1/max(denom, 1e-9)
                den = wp.tile([128, 4], f32, tag="den")
                nc.vector.tensor_reduce(den[:, :, None], ex1[:], op=OP.add,
                                        axis=mybir.AxisListType.X)
                eps_b = bass.AP(epst.tensor, epst[:].offset,
                                [epst[:].ap[0], [0, 4]])
                nc.vector.tensor_tensor(den[:], den[:], eps_b, OP.max)
                rd = wp.tile([128, 4], f32, tag="rd")
                nc.vector.reciprocal(rd[:], den[:])
                # alpha = ex * rd_bcast  (fp16, h-major)
                alpha = wp.tile([128, 4, Jtp], f16, tag="alpha")
                rd_b = bass.AP(rd.tensor, rd[:].offset,
                               [rd[:].ap[0], [1, 4], [0, Jtp]])
                nc.vector.tensor_tensor(alpha[:], ex1[:], rd_b, OP.mult)
                # msg = alpha * ft, in place into g's ft cols
                D = GC // 4
                al_b = bass.AP(alpha.tensor, alpha[:].offset,
                               [alpha[:].ap[0], [1, Jtp], [Jtp, 4], [0, D]])
                ft4 = bass.AP(g.tensor, g[:].offset,
                              [g[:].ap[0], [TC, Jtp], [D, 4], [1, D]])
                nc.vector.tensor_tensor(ft4, ft4, al_b, OP.mult)
                # out_row = sum_j msg  (halving tree over j, in place)
                ro = wp.tile([128, GC], f32, tag="ro")

                def jsl(j0, cnt):
                    return bass.AP(g.tensor, g[:].offset + j0 * TC,
                                   [g[:].ap[0], [TC, cnt], [1, GC]])

                n = Jtp
                while n > 2:
                    h = n // 2
                    nc.vector.tensor_tensor(jsl(0, h), jsl(0, h),
                                            jsl(n - h, h), OP.add)
                    n -= h
                if n == 2:
                    nc.vector.tensor_tensor(ro[:], jsl(0, 1), jsl(1, 1),
                                            OP.add)
                else:
                    nc.vector.tensor_copy(ro[:], jsl(0, 1))
                if l < 2:
                    # h = elu(ro) = max(ro, min(exp(ro),1)-1); -> fp16 -> hT
                    # (tensor_tensor only — DVE tensor_scalar would block
                    # GpSimd descriptor generation)
                    ev = wp.tile([128, GC], f32, tag="ev")
                    nc.scalar.activation(ev[:], ro[:], AF.Exp)
                    one_b = bass.AP(ones_t.tensor, ones_t[:].offset,
                                    [ones_t[:].ap[0], [0, GC]])
                    nc.vector.tensor_tensor(ev[:], ev[:], one_b, OP.min)
                    nc.vector.tensor_tensor(ev[:], ev[:], one_b, OP.subtract)
                    nc.vector.tensor_tensor(ro[:], ro[:], ev[:], OP.max)
                    for k in range(2):
                        tp = tps.tile([128, 128], f32, tag="trps")
                        nc.tensor.transpose(
                            tp[:], ro[:, k * 128:(k + 1) * 128], identf[:])
                        nc.scalar.activation(hT[k][p], tp[:], AF.Copy)
                else:
                    # logits = mean over heads (1/H folded into W2e ft);
                    # log_softmax
                    z = wp.tile([128, NCLS], f32, tag="z")
                    ro_h = bass.AP(ro.tensor, ro[:].offset,
                                   [ro[:].ap[0], [1, NCLS], [NCLS, 4]])
                    z_v = bass.AP(z.tensor, z[:].offset,
                                  [z[:].ap[0], [1, NCLS], [0, 1]])
                    nc.vector.tensor_reduce(z_v, ro_h, op=OP.add,
                                            axis=mybir.AxisListType.X)
                    m = wp.tile([128, 1], f32, tag="m")
                    nc.vector.tensor_reduce(m[:], z[:], op=OP.max,
                                            axis=mybir.AxisListType.X)
                    nm = wp.tile([128, 1], f32, tag="nm")
                    nc.scalar.activation(nm[:], m[:], AF.Copy, scale=-1.0)
                    ez = wp.tile([128, NCLS], f32, tag="ez")
                    s = wp.tile([128, 1], f32, tag="s")
                    nc.scalar.activation(ez[:], z[:], AF.Exp, bias=nm[:],
                                         accum_out=s[:])
                    lns = wp.tile([128, 1], f32, tag="lns")
                    nc.scalar.activation(lns[:], s[:], AF.Ln)
                    b = wp.tile([128, 1], f32, tag="b")
                    nc.vector.tensor_tensor(b[:], m[:], lns[:], OP.add)
                    lp = wp.tile([128, NCLS], f32, tag="lp")
                    b_b = bass.AP(b.tensor, b[:].offset,
                                  [b[:].ap[0], [0, NCLS]])
                    nc.vector.tensor_tensor(lp[:], z[:], b_b, OP.subtract)
                    nc.sync.dma_start(out_d[p * 128:(p + 1) * 128, :], lp[:])

            # sweep 0: node phase of layer 0 (+ its chunked all-gathers)
            for p in ORDER:
                node_phase(0, p)
            # sweeps 1,2: edge(l-1) interleaved with node(l)
            for l in (1, 2):
                for p in ORDER:
                    edge_gather(l - 1, p)
                for p in ORDER:
                    edge_phase(l - 1, p)
                    node_phase(l, p)
            # sweep 3: edge phase of layer 2 -> logits
            for p in ORDER:
                edge_gather(2, p)
            for p in ORDER:
                edge_phase(2, p)
    nc.compile()
    return nc


def _install_trace_shim():
    """Provide antenv.axon_hooks (get/set NTFF profile hook) when absent."""
    import sys, types
    try:
        from antenv.axon_hooks import get_axon_ntff_profile_hook  # noqa
        return
    except ImportError:
        pass
    mod = types.ModuleType("antenv.axon_hooks")
    _hook = [None]
    mod.set_axon_ntff_profile_hook = lambda h: _hook.__setitem__(0, h)
    mod.get_axon_ntff_profile_hook = lambda: _hook[0]
    sys.modules["antenv.axon_hooks"] = mod
    import antenv
    antenv.axon_hooks = mod
    if "/root/.axon_site" not in sys.path:
        sys.path.insert(0, "/root/.axon_site")
    from trn_agent_boot.trn_boot import _ntff_profile_via_ctypes
    mod.set_axon_ntff_profile_hook(
        _ntff_profile_via_ctypes("/opt/axon/libaxon_pjrt.so"))


def kernel(features, src, dst, W0, al0, ar0, W1, al1, ar1, W2, al2, ar2):
    import sys, os
    for pth in ("/root/axon_fix", "/opt/trn_rl_repo"):
        if os.path.isdir(pth) and pth not in sys.path:
            sys.path.insert(0, pth)
    if os.environ.get("KERNEL_TRACE"):
        _install_trace_shim()
    from concourse import bass_utils

    src = np.asarray(src).astype(np.int64)
    dst = np.asarray(dst).astype(np.int64)
    features = np.asarray(features, np.float32)

    core_of, pos_of, slot_of = _pack_nodes(src, dst)
    pos_of, ACH, BCH, ORDER = _reorder_positions(src, dst, core_of, pos_of,
                                                 slot_of)
    _evict_reserved(core_of, pos_of, slot_of)
    idxT, J_a, J_b, colbase = _build_grids(src, dst, core_of, pos_of, slot_of,
                                           ACH, BCH)
    Wes = [_fold_weights(np.asarray(W0, np.float32), np.asarray(al0), np.asarray(ar0)),
           _fold_weights(np.asarray(W1, np.float32), np.asarray(al1), np.asarray(ar1)),
           _fold_weights(np.asarray(W2, np.float32), np.asarray(al2), np.asarray(ar2),
                         ft_scale=1.0 / HEADS)]

    # per-core featT [256, SLAB] fp16 in slot order
    featTs = []
    for c in range(NCORES):
        ft = np.zeros((256, NPOS * 128), np.float16)
        mask = core_of == c
        ids = np.arange(N_NODES)[mask]
        cols = pos_of[ids] * 128 + slot_of[ids]
        ft[:, cols] = features[ids].T.astype(np.float16)
        featTs.append(ft)

    nc = _build_program(J_a, J_b, colbase, ACH, BCH, ORDER)
    ins = [{"featT": featTs[c], "idxT": idxT[c],
            "W0e": Wes[0], "W1e": Wes[1], "W2e": Wes[2]}
           for c in range(NCORES)]
    res = bass_utils.run_bass_kernel_spmd(
        nc, ins, core_ids=list(range(NCORES)),
        trace=bool(os.environ.get("KERNEL_TRACE")))
    if os.environ.get("KERNEL_TRACE"):
        print("HW exec time:", res.exec_time_ns, "ns")
        kernel.last_exec_ns = res.exec_time_ns
        kernel.last_trace = res.instructions_and_trace

    out = np.empty((N_NODES, NCLS), np.float32)
    for c in range(NCORES):
        mask = core_of == c
        ids = np.arange(N_NODES)[mask]
        rows = pos_of[ids] * 128 + slot_of[ids]
        out[ids] = res.results[c]["out"][rows]
    return out



# revision 25
# speedup vs baseline: 1.2762x; 1.0733x over previous
"""GAT 3-layer kernel for 8 TRN2 NeuronCores (slot-major edge-parallel design).

Sharding: dst nodes packed into 392 blocks of 128 slots (one slot = one SBUF
partition), blocks dealt to 8 cores x 49 positions. Edges live on the free dim
of each block (j-slabs), split lo/hi by source-table row (<32768 vs >=) so the
int16 dma_gather indices stay in range. Node features/tables are all-gathered
per layer (chunked, overlapped with the node phase); everything else is local.
"""
import numpy as np

N_NODES = 50000
E_EDGES = 800000
IN_FEATS = 256
HID = 64
HEADS = 4
NCLS = 40
NEG_SLOPE = 0.2

NCORES = 8
NPOS = 49                    # blocks per core
NPA = 32                     # positions in table A (rest in table B)
SLAB_A = NPA * 128           # 4096 rows/core in table A
SLAB_B = (NPOS - NPA) * 128  # 2176 rows/core in table B
ROWS_A = NCORES * SLAB_A     # 32768
ROWS_B = NCORES * SLAB_B     # 17408
A_SIZE = ROWS_A - 8          # A-region real-node capacity (8 reserved slots)
DUMMY_A_ROW = ROWS_A - 1     # (core 7, pos 31, slot 127)
DUMMY_B_ROW = ROWS_B - 1     # (core 7, pos 48, slot 127)
RES_POS = (NPA - 1, NPOS - 1)  # slot 127 reserved on every core

def _srow_of(core, pos, slot, ach, bch):
    """Table row for a node at (core, pos, slot) under chunk-major layout."""
    pos = np.asarray(pos)
    out = np.zeros(pos.shape, np.int64)
    base = 0
    for (lo, hi) in ach:
        nrow = (hi - lo) * 128
        m = (pos >= lo) & (pos < hi)
        out[m] = base + core[m] * nrow + (pos[m] - lo) * 128 + slot[m]
        base += NCORES * nrow
    base = 0
    for (lo, hi) in bch:
        nrow = (hi - lo) * 128
        m = (pos >= lo) & (pos < hi)
        out[m] = base + core[m] * nrow + (pos[m] - lo) * 128 + slot[m]
        base += NCORES * nrow
    return out


def _reorder_positions(src, dst, core_of, pos_of, slot_of):
    """Relabel positions so process order (=label order per region) is a
    light-heavy-light pyramid: early labels are light (their AG chunks fire
    early in the sweep), heavies sit mid-sweep, and the final label of each
    region is light so the tail AllGather gating the next sweep is tiny.
    Returns (new pos_of, ACH, BCH, ORDER)."""
    in_b_node = pos_of >= NPA
    na = np.zeros(NPOS, np.int64)
    nb = np.zeros(NPOS, np.int64)
    # per-(core,pos,slot) counts of A/B edges -> per-pos max (the J profile)
    eb = (pos_of[src] >= NPA).astype(np.int64)
    key = (core_of[dst] * NPOS + pos_of[dst]) * 128 + slot_of[dst]
    for v, arr in ((0, na), (1, nb)):
        m = eb == v
        cnt = np.bincount(key[m], minlength=NCORES * NPOS * 128)
        mx = cnt.reshape(NCORES, NPOS, 128).max(axis=(0, 2))
        arr[:] = mx
    jt = np.maximum(na, 1) + np.maximum(nb, 1)

    def pyramid(labels):
        asc = sorted(labels, key=lambda p: jt[p])
        return asc[0::2] + asc[1::2][::-1]

    seqA = pyramid(list(range(NPA)))          # old labels, process order
    seqB = pyramid(list(range(NPA, NPOS)))
    # relabel: process order becomes label order within each region
    newpos = np.empty(NPOS, np.int64)
    for i, op in enumerate(seqA):
        newpos[op] = i
    for i, op in enumerate(seqB):
        newpos[op] = NPA + i
    pos_of2 = newpos[pos_of]
    jtA = jt[seqA]
    jtB = jt[seqB]

    def chunks(jts, lo0, fracs):
        cum = np.cumsum(jts) / jts.sum()
        n = len(jts)
        bnds = [lo0]
        for f in fracs:
            b = lo0 + int(np.searchsorted(cum, f) + 1)
            if b > bnds[-1] and b < lo0 + n - 1:
                bnds.append(b)
        if bnds[-1] != lo0 + n - 1:
            bnds.append(lo0 + n - 1)      # last chunk = single light position
        bnds.append(lo0 + n)
        return [(bnds[i], bnds[i + 1]) for i in range(len(bnds) - 1)]

    ach = chunks(jtA, 0, [0.33, 0.62, 0.86])
    bch = chunks(jtB, NPA, [0.5, 0.84])
    # merged process order: interleave A and B labels by cumulative fraction
    cumA = np.cumsum(jtA) / jtA.sum()
    cumB = np.cumsum(jtB) / jtB.sum()
    tagged = [(cumA[i], i) for i in range(NPA)] + \
             [(cumB[i], NPA + i) for i in range(NPOS - NPA)]
    order = [p for _, p in sorted(tagged)]
    return pos_of2, ach, bch, order


def _pack_nodes(src, dst):
    """Assign each node a (core, pos, slot); A-set = ids < A_SIZE -> pos < NPA.

    2D-banded packing: band by one in-degree coordinate, sort by the other
    within each band, so each 1024-node position block is homogeneous in
    (n_a, n_b) and per-block maxima (the slab padding) stay small."""
    deg = np.bincount(dst, minlength=N_NODES)
    n_a = np.bincount(dst[src < A_SIZE], minlength=N_NODES)
    n_b = deg - n_a

    ids = np.arange(N_NODES)
    a_ids = ids[:A_SIZE]
    b_ids = ids[A_SIZE:]

    def banded(idset, pri, sec, G):
        o = idset[np.argsort(-pri[idset], kind='stable')]
        out = []
        for i in range(0, len(o), 1024 * G):
            band = o[i:i + 1024 * G]
            out.append(band[np.argsort(-sec[band], kind='stable')])
        return np.concatenate(out)

    def chunk_cost(order, npos):
        tot, k = 0, 0
        for p in range(npos):
            take = order[k:k + 1024]
            k += len(take)
            tot += max(n_a[take].max(), 1) + max(n_b[take].max(), 1)
        return tot

    def best(idset, npos):
        cands = []
        for G in (2, 3, 4, 5, 6):
            for pri, sec in ((n_a, n_b), (n_b, n_a)):
                o = banded(idset, pri, sec, G)
                cands.append((chunk_cost(o, npos), o))
        cands.sort(key=lambda t: t[0])
        return cands[0][1]

    a_sorted = best(a_ids, NPA)
    b_sorted = best(b_ids, NPOS - NPA)

    core_of = np.full(N_NODES, -1, np.int32)
    pos_of = np.full(N_NODES, -1, np.int32)
    slot_of = np.full(N_NODES, -1, np.int32)

    def fill(sorted_ids, pos0):
        # snake deal: alternate core direction per position so the
        # degree-sorted blocks spread evenly across cores
        k = 0
        pos, ci = pos0, 0
        while k < len(sorted_ids):
            take = sorted_ids[k:k + 128]
            k += len(take)
            c = ci if (pos % 2 == 0) else NCORES - 1 - ci
            core_of[take] = c
            pos_of[take] = pos
            slot_of[take] = np.arange(len(take))
            ci += 1
            if ci == NCORES:
                ci = 0
                pos += 1

    fill(a_sorted, 0)
    fill(b_sorted, NPA)
    return core_of, pos_of, slot_of


def _evict_reserved(core_of, pos_of, slot_of):
    """After relabeling, slot 127 of the RES_POS positions (labels NPA-1 and
    NPOS-1, every core) must be node-free: the node phase stamps the dummy
    el marker (-80) on partition 127 there. Move any occupants to free
    slots elsewhere in their region."""
    occ = np.zeros((NCORES, NPOS, 128), bool)
    occ[core_of, pos_of, slot_of] = True
    for rp, lo, hi in ((NPA - 1, 0, NPA), (NPOS - 1, NPA, NPOS)):
        offenders = np.where((pos_of == rp) & (slot_of == 127))[0]
        free = np.argwhere(~occ[:, lo:hi, :])
        free = [(c, lo + p, s) for c, p, s in free
                if not (p + lo == rp and s == 127)]
        assert len(free) >= len(offenders)
        for i, node in enumerate(offenders):
            c, p, s = free[i]
            occ[core_of[node], pos_of[node], slot_of[node]] = False
            core_of[node], pos_of[node], slot_of[node] = c, p, s
            occ[c, p, s] = True


def _build_grids(src, dst, core_of, pos_of, slot_of, ach, bch):
    """Per-core wrapped int16 idx arrays + per-position J_a/J_b schedules."""
    in_b = (pos_of[src] >= NPA).astype(np.int64)
    srow = _srow_of(core_of[src], pos_of[src], slot_of[src], ach, bch)
    dcore = core_of[dst]
    dpos = pos_of[dst]
    dslot = slot_of[dst]
    drow = dcore * (NPOS * 128) + dpos * 128 + dslot

    order = np.lexsort((srow, in_b, drow))
    gkey = drow[order] * 2 + in_b[order]
    newgrp = np.concatenate([[True], gkey[1:] != gkey[:-1]])
    gstart = np.maximum.accumulate(np.where(newgrp, np.arange(E_EDGES), 0))
    rank = np.arange(E_EDGES) - gstart
    j_in = np.empty(E_EDGES, np.int64)
    j_in[order] = rank

    na_e = np.where(in_b == 0, j_in + 1, 0)
    nb_e = np.where(in_b == 1, j_in + 1, 0)
    J_a = np.zeros(NPOS, np.int64)
    J_b = np.zeros(NPOS, np.int64)
    np.maximum.at(J_a, dpos, na_e)
    np.maximum.at(J_b, dpos, nb_e)
    J_a = np.maximum(J_a, 1)
    J_b = np.maximum(J_b, 1)

    grids = [[np.concatenate([
                np.full((J_a[p], 128), DUMMY_A_ROW, np.int64),
                np.full((J_b[p], 128), DUMMY_B_ROW, np.int64)])
              for p in range(NPOS)] for c in range(NCORES)]
    j_eff = np.where(in_b == 0, j_in, J_a[dpos] + j_in)
    for e in range(E_EDGES):
        grids[dcore[e]][dpos[e]][j_eff[e], dslot[e]] = srow[e]

    def wrap(grid):
        flat = grid.reshape(-1)
        w = flat.reshape(-1, 8, 16).transpose(2, 0, 1).reshape(16, -1)
        return np.tile(w, (8, 1)).astype(np.int16)

    idxT = []
    colbase = []
    for c in range(NCORES):
        parts = []
        cb = []
        col = 0
        for p in range(NPOS):
            a_w = wrap(grids[c][p][:J_a[p]])
            b_w = wrap(grids[c][p][J_a[p]:])
            cb.append((col, col + a_w.shape[1]))
            col += a_w.shape[1] + b_w.shape[1]
            parts.append(a_w)
            parts.append(b_w)
        idxT.append(np.concatenate(parts, axis=1))
        colbase = cb
    return idxT, J_a.tolist(), J_b.tolist(), colbase


def _fold_weights(W, al, ar, ft_scale=1.0):
    """[W | A | B] with A[k,h]=sum_d W[k,h*D+d]*al[h,d] (el), B likewise (er).
    ft_scale scales only the W (message) part — used to fold the final
    mean-over-heads 1/H into layer 2's ft."""
    H, D = al.shape
    Wr = W.reshape(W.shape[0], H, D)
    A = np.einsum('khd,hd->kh', Wr, al)
    B = np.einsum('khd,hd->kh', Wr, ar)
    return np.concatenate([W * ft_scale, A, B], axis=1).astype(np.float16)


def _build_program(J_a, J_b, colbase, ACH, BCH, ORDER):
    import concourse.bacc as bacc
    import concourse.bass as bass
    import concourse.mybir as mybir
    from concourse.tile import TileContext

    f16 = mybir.dt.float16
    f32 = mybir.dt.float32
    AF = mybir.ActivationFunctionType
    OP = mybir.AluOpType

    TCOLS = [384, 384, 256]      # fp16 cols per table row per layer (stride)
    GCOLS = [256, 256, 160]      # ft cols per layer
    CCOLS = [264, 264, 168]      # node-phase out cols (ft + el4 + er4)
    TCC = [264, 264, 168]        # compact staged cols (ft + el4-as-f32)
    TOTC = colbase[-1][1] + J_b[-1] * 8
    Jt = [J_a[p] + J_b[p] for p in range(NPOS)]

    # Heavy positions are split into parts of <= MAXJ slabs so every g tile
    # comes from one uniform deep ring (6 bufs): small tiles keep SBUF in
    # budget while the deep ring keeps all 4 gather queues fed ahead of the
    # vector engine.
    MAXJ = 24
    GS_BUFS = 6
    PARTS = []
    for p in range(NPOS):
        if Jt[p] <= MAXJ:
            PARTS.append([(0, Jt[p])])
        else:
            m = Jt[p] // 2
            PARTS.append([(0, m), (m, Jt[p])])

    nc = bacc.Bacc("TRN2", num_devices=NCORES, num_swdge_queues=4)
    featT = nc.dram_tensor("featT", [256, NPOS * 128], f16, kind="ExternalInput")
    idxT = nc.dram_tensor("idxT", [128, TOTC], mybir.dt.int16, kind="ExternalInput")
    Wes = [nc.dram_tensor(f"W{l}e", [256, CCOLS[l]], f16, kind="ExternalInput")
           for l in range(3)]
    out_d = nc.dram_tensor("out", [NPOS * 128, NCLS], f32, kind="ExternalOutput")

    agins = []   # per layer: list of (pos_lo, pos_hi, is_b, row_base, tensor)
    for l in range(3):
        chunks = []
        base = 0
        for (lo, hi) in ACH:
            t = nc.dram_tensor(f"aga{l}_{lo}", [(hi - lo) * 128, TCOLS[l]],
                               f16, kind="Internal")
            chunks.append((lo, hi, False, base, t))
            base += NCORES * (hi - lo) * 128
        base = 0
        for (lo, hi) in BCH:
            t = nc.dram_tensor(f"agb{l}_{lo}", [(hi - lo) * 128, TCOLS[l]],
                               f16, kind="Internal")
            chunks.append((lo, hi, True, base, t))
            base += NCORES * (hi - lo) * 128
        agins.append(chunks)
    table_a = [nc.dram_tensor(f"tablea{l}", [ROWS_A, TCOLS[l]], f16,
                              kind="Internal", addr_space="Shared")
               for l in range(3)]
    table_b = [nc.dram_tensor(f"tableb{l}", [ROWS_B, TCOLS[l]], f16,
                              kind="Internal", addr_space="Shared")
               for l in range(3)]

    qn = [0]

    def next_q():
        qn[0] = (qn[0] + 1) % 4
        return qn[0]

    with TileContext(nc) as tc:
        with tc.tile_pool(name="resident", bufs=1) as rp, \
             tc.tile_pool(name="work", bufs=4) as wp, \
             tc.tile_pool(name="gather", bufs=2) as gp, \
             tc.tile_pool(name="nps", bufs=3, space="PSUM") as nps, \
             tc.tile_pool(name="tps", bufs=4, space="PSUM") as tps:

            ia = rp.tile([128, TOTC], mybir.dt.int16)
            nc.sync.dma_start(ia[:], idxT[:])
            NP128 = NPOS * 128
            hTbig = rp.tile([128, 2 * NP128], f16)
            nc.sync.dma_start(hTbig[:, :NP128], featT[0:128, :])
            nc.sync.dma_start(hTbig[:, NP128:], featT[128:256, :])
            hT = [[bass.AP(hTbig.tensor,
                           hTbig[:].offset + k * NP128 + p * 128,
                           [hTbig[:].ap[0], [1, 128]])
                   for p in range(NPOS)] for k in range(2)]
            er_own = [rp.tile([128, 4], f32, tag=f"er{p}", name=f"er{p}")
                      for p in range(NPOS)]
            Wt = [rp.tile([128, 2, CCOLS[l]], f16, tag=f"Wt{l}", name=f"Wt{l}")
                  for l in range(3)]
            for l in range(3):
                nc.sync.dma_start(
                    Wt[l][:], Wes[l][:].rearrange("(k p) n -> p k n", k=2))
            # fp32 identity for PE transpose
            colv = rp.tile([128, 128], mybir.dt.int32)
            nc.gpsimd.iota(colv[:], [[1, 128]], base=0, channel_multiplier=0)
            rowv = rp.tile([128, 1], mybir.dt.int32)
            nc.gpsimd.iota(rowv[:], [[0, 1]], base=0, channel_multiplier=1)
            row_b = bass.AP(rowv.tensor, rowv[:].offset,
                            [rowv[:].ap[0], [0, 128]])
            identf = rp.tile([128, 128], f32)
            nc.vector.tensor_tensor(identf[:], colv[:], row_b, OP.is_equal)
            # -80 at partition 127, 0 elsewhere (dummy-row el marker)
            dmask = rp.tile([128, 1], f32)
            nc.vector.tensor_scalar(dmask[:], rowv[:], 127, -80.0,
                                    OP.is_equal, OP.mult)
            # const tiles for contention-free tensor_tensor broadcasts
            ones_t = rp.tile([128, 1], f32)
            nc.vector.memset(ones_t[:], 1.0)

            def node_phase(l, p):
                # copies/casts run on the scalar engine: DVE copy/cast ops
                # enter 2-port perf mode and fully block GpSimd SWDGE
                # descriptor generation (ACT never contends).
                GC, CC = GCOLS[l], CCOLS[l]
                ps = nps.tile([128, CC], f32, tag="nodeps")
                for k in range(2):
                    nc.tensor.matmul(
                        ps[:], hT[k][p],
                        Wt[l][:].rearrange("p k n -> k p n")[k],
                        start=(k == 0), stop=(k == 1))
                nc.scalar.activation(er_own[p][:], ps[:, GC + 4:GC + 8],
                                     AF.Copy)
                stage = wp.tile([128, TCOLS[l]], f16, tag="stage")
                nc.scalar.activation(stage[:, :GC], ps[:, :GC], AF.Copy)
                st32 = stage[:].bitcast(f32)
                if p in RES_POS:
                    dm_b = bass.AP(dmask.tensor, dmask[:].offset,
                                   [dmask[:].ap[0], [0, 4]])
                    nc.vector.tensor_tensor(st32[:, GC // 2:GC // 2 + 4],
                                            ps[:, GC:GC + 4], dm_b, OP.add)
                else:
                    nc.scalar.activation(st32[:, GC // 2:GC // 2 + 4],
                                         ps[:, GC:GC + 4], AF.Copy)
                # stage -> the agin chunk containing position p (scalar-queue
                # HWDGE so the sync queue stays free)
                for (lo, hi, is_b, rb, t) in agins[l]:
                    if lo <= p < hi:
                        nc.scalar.dma_start(
                            t[(p - lo) * 128:(p - lo + 1) * 128, :], stage[:])
                # fire the collective for any chunk that just completed.
                # Fine-grained chunks keep each trigger's stage-DMA wait
                # short, so the in-order GpSimd queue is only briefly
                # blocked; the tiny last chunk keeps the AG that gates the
                # next sweep's gathers off the critical path.
                for (lo, hi, is_b, rb, t) in agins[l]:
                    if p == hi - 1:
                        tab = table_b[l] if is_b else table_a[l]
                        nrow = (hi - lo) * 128
                        nc.gpsimd.collective_compute(
                            "AllGather", OP.bypass,
                            replica_groups=[list(range(NCORES))],
                            ins=[t[:].opt()],
                            outs=[tab[rb:rb + NCORES * nrow, :].opt()])

            g_tiles = {}

            def edge_gather(l, p):
                # each part -> its own tile; pieces split across the 4 SWDGE
                # queues (gather throughput is per-row per-queue)
                GC, TC = GCOLS[l], TCOLS[l]
                Jl, Jh = J_a[p], J_b[p]
                locol, hicol = colbase[p]
                tiles = []
                for (j0, j1) in PARTS[p]:
                    nj = j1 - j0
                    g = gp.tile([128, nj, 384], f16, tag="gs", bufs=GS_BUFS,
                                name="g")
                    tiles.append(g)

                    def gg(table, col0, nj2, dj):
                        # sub-split big pieces across two queues
                        if nj2 >= 6:
                            h = nj2 // 2
                            pieces = [(0, h), (h, nj2)]
                        else:
                            pieces = [(0, nj2)]
                        for (a, b) in pieces:
                            nc.gpsimd.dma_gather(
                                bass.AP(g.tensor,
                                        g[:].offset + (dj + a) * TC,
                                        [g[:].ap[0], [TC, b - a], [1, TC]]),
                                table[:, :],
                                ia[:, col0 + a * 8:col0 + b * 8],
                                (b - a) * 128, (b - a) * 128, TC,
                                single_packet=False, queue_num=next_q())

                    # A-piece of this part: slabs [j0, min(j1, Jl))
                    if j0 < Jl:
                        s0, s1 = j0, min(j1, Jl)
                        gg(table_a[l], locol + s0 * 8, s1 - s0, 0)
                    # B-piece: slabs [max(j0, Jl), j1)
                    if j1 > Jl:
                        s0, s1 = max(j0, Jl), j1
                        gg(table_b[l], hicol + (s0 - Jl) * 8, s1 - s0,
                           s0 - j0)
                g_tiles[p] = tiles

            def edge_phase(l, p):
                GC, TC = GCOLS[l], TCOLS[l]
                tiles = g_tiles.pop(p)
                parts = PARTS[p]
                # per part: e = el + er; ex = exp(lrelu(e)) on scalar;
                # den_part = sum_j ex
                exs, dens = [], []
                for g, (j0, j1) in zip(tiles, parts):
                    nj = j1 - j0
                    elv = bass.AP(g.tensor, g[:].offset,
                                  [g[:].ap[0], [TC, nj], [1, TC]]
                                  ).bitcast(f32)
                    el_hm = bass.AP(elv.tensor, elv.offset + GC // 2,
                                    [elv.ap[0], [1, 4], [TC // 2, nj]])
                    e_t = wp.tile([128, 4, nj], f32, tag="e")
                    er_b = bass.AP(er_own[p].tensor, er_own[p][:].offset,
                                   [er_own[p][:].ap[0], [1, 4], [0, nj]])
                    nc.vector.tensor_tensor(e_t[:], el_hm, er_b, OP.add)
                    ex1 = wp.tile([128, 4, nj], f32, tag="ex1")
                    nc.scalar.activation(ex1[:], e_t[:], AF.Lrelu,
                                         alpha=NEG_SLOPE)
                    nc.scalar.activation(ex1[:], ex1[:], AF.Exp)
                    den = wp.tile([128, 4], f32, tag="den")
                    nc.vector.tensor_reduce(den[:, :, None], ex1[:],
                                            op=OP.add,
                                            axis=mybir.AxisListType.X)
                    exs.append(ex1)
                    dens.append(den)
                if len(dens) > 1:
                    nc.vector.tensor_tensor(dens[0][:], dens[0][:],
                                            dens[1][:], OP.add)
                rd = wp.tile([128, 4], f32, tag="rd")
                nc.vector.reciprocal(rd[:], dens[0][:])
                # per part: alpha = ex*rd (fp16); msg = alpha*ft in place;
                # tree-sum msg over j -> ro (accumulated across parts)
                D = GC // 4
                ro = wp.tile([128, GC], f32, tag="ro")
                for pi, (g, (j0, j1)) in enumerate(zip(tiles, parts)):
                    nj = j1 - j0
                    ex1 = exs[pi]
                    alpha = wp.tile([128, 4, nj], f16, tag="alpha")
                    rd_b = bass.AP(rd.tensor, rd[:].offset,
                                   [rd[:].ap[0], [1, 4], [0, nj]])
                    nc.vector.tensor_tensor(alpha[:], ex1[:], rd_b, OP.mult)
                    al_b = bass.AP(alpha.tensor, alpha[:].offset,
                                   [alpha[:].ap[0], [1, nj], [nj, 4], [0, D]])
                    ft4 = bass.AP(g.tensor, g[:].offset,
                                  [g[:].ap[0], [TC, nj], [D, 4], [1, D]])
                    nc.vector.tensor_tensor(ft4, ft4, al_b, OP.mult)

                    def jsl(j0c, cnt):
                        return bass.AP(g.tensor, g[:].offset + j0c * TC,
                                       [g[:].ap[0], [TC, cnt], [1, GC]])

                    n = nj
                    while n > 2:
                        h = n // 2
                        nc.vector.tensor_tensor(jsl(0, h), jsl(0, h),
                                                jsl(n - h, h), OP.add)
                        n -= h
                    assert n == 2, nj
                    if pi == 0:
                        nc.vector.tensor_tensor(ro[:], jsl(0, 1),
                                                jsl(1, 1), OP.add)
                    else:
                        nc.vector.tensor_tensor(jsl(0, 1), jsl(0, 1),
                                                jsl(1, 1), OP.add)
                        nc.vector.tensor_tensor(ro[:], ro[:], jsl(0, 1),
                                                OP.add)
                if l < 2:
                    # h = elu(ro) = max(ro, min(exp(ro),1)-1); -> fp16 -> hT
                    # (tensor_tensor only — DVE tensor_scalar would block
                    # GpSimd descriptor generation)
                    ev = wp.tile([128, GC], f32, tag="ev")
                    nc.scalar.activation(ev[:], ro[:], AF.Exp)
                    one_b = bass.AP(ones_t.tensor, ones_t[:].offset,
                                    [ones_t[:].ap[0], [0, GC]])
                    nc.vector.tensor_tensor(ev[:], ev[:], one_b, OP.min)
                    nc.vector.tensor_tensor(ev[:], ev[:], one_b, OP.subtract)
                    nc.vector.tensor_tensor(ro[:], ro[:], ev[:], OP.max)
                    for k in range(2):
                        tp = tps.tile([128, 128], f32, tag="trps")
                        nc.tensor.transpose(
                            tp[:], ro[:, k * 128:(k + 1) * 128], identf[:])
                        nc.scalar.activation(hT[k][p], tp[:], AF.Copy)
                else:
                    # logits = mean over heads (1/H folded into W2e ft);
                    # log_softmax
                    z = wp.tile([128, NCLS], f32, tag="z")
                    ro_h = bass.AP(ro.tensor, ro[:].offset,
                                   [ro[:].ap[0], [1, NCLS], [NCLS, 4]])
                    z_v = bass.AP(z.tensor, z[:].offset,
                                  [z[:].ap[0], [1, NCLS], [0, 1]])
                    nc.vector.tensor_reduce(z_v, ro_h, op=OP.add,
                                            axis=mybir.AxisListType.X)
                    m = wp.tile([128, 1], f32, tag="m")
                    nc.vector.tensor_reduce(m[:], z[:], op=OP.max,
                                            axis=mybir.AxisListType.X)
                    nm = wp.tile([128, 1], f32, tag="nm")
                    nc.scalar.activation(nm[:], m[:], AF.Copy, scale=-1.0)
                    ez = wp.tile([128, NCLS], f32, tag="ez")
                    s = wp.tile([128, 1], f32, tag="s")
                    nc.scalar.activation(ez[:], z[:], AF.Exp, bias=nm[:],
                                         accum_out=s[:])
                    lns = wp.tile([128, 1], f32, tag="lns")
                    nc.scalar.activation(lns[:], s[:], AF.Ln)
                    b = wp.tile([128, 1], f32, tag="b")
                    nc.vector.tensor_tensor(b[:], m[:], lns[:], OP.add)
                    lp = wp.tile([128, NCLS], f32, tag="lp")
                    b_b = bass.AP(b.tensor, b[:].offset,
                                  [b[:].ap[0], [0, NCLS]])
                    nc.vector.tensor_tensor(lp[:], z[:], b_b, OP.subtract)
                    nc.sync.dma_start(out_d[p * 128:(p + 1) * 128, :], lp[:])

            # sweep 0: node phase of layer 0 (+ its chunked all-gathers)
            for p in ORDER:
                node_phase(0, p)
            # sweeps 1,2: edge(l-1) interleaved with node(l)
            for l in (1, 2):
                for p in ORDER:
                    edge_gather(l - 1, p)
                for p in ORDER:
                    edge_phase(l - 1, p)
                    node_phase(l, p)
            # sweep 3: edge phase of layer 2 -> logits
            for p in ORDER:
                edge_gather(2, p)
            for p in ORDER:
                edge_phase(2, p)
    nc.compile()
    return nc


def _install_trace_shim():
    """Provide antenv.axon_hooks (get/set NTFF profile hook) when absent."""
    import sys, types
    try:
        from antenv.axon_hooks import get_axon_ntff_profile_hook  # noqa
        return
    except ImportError:
        pass
    mod = types.ModuleType("antenv.axon_hooks")
    _hook = [None]
    mod.set_axon_ntff_profile_hook = lambda h: _hook.__setitem__(0, h)
    mod.get_axon_ntff_profile_hook = lambda: _hook[0]
    sys.modules["antenv.axon_hooks"] = mod
    import antenv
    antenv.axon_hooks = mod
    if "/root/.axon_site" not in sys.path:
        sys.path.insert(0, "/root/.axon_site")
    from trn_agent_boot.trn_boot import _ntff_profile_via_ctypes
    mod.set_axon_ntff_profile_hook(
        _ntff_profile_via_ctypes("/opt/axon/libaxon_pjrt.so"))


def kernel(features, src, dst, W0, al0, ar0, W1, al1, ar1, W2, al2, ar2):
    import sys, os
    for pth in ("/root/axon_fix", "/opt/trn_rl_repo"):
        if os.path.isdir(pth) and pth not in sys.path:
            sys.path.insert(0, pth)
    if os.environ.get("KERNEL_TRACE"):
        _install_trace_shim()
    from concourse import bass_utils

    src = np.asarray(src).astype(np.int64)
    dst = np.asarray(dst).astype(np.int64)
    features = np.asarray(features, np.float32)

    core_of, pos_of, slot_of = _pack_nodes(src, dst)
    pos_of, ACH, BCH, ORDER = _reorder_positions(src, dst, core_of, pos_of,
                                                 slot_of)
    _evict_reserved(core_of, pos_of, slot_of)
    idxT, J_a, J_b, colbase = _build_grids(src, dst, core_of, pos_of, slot_of,
                                           ACH, BCH)
    Wes = [_fold_weights(np.asarray(W0, np.float32), np.asarray(al0), np.asarray(ar0)),
           _fold_weights(np.asarray(W1, np.float32), np.asarray(al1), np.asarray(ar1)),
           _fold_weights(np.asarray(W2, np.float32), np.asarray(al2), np.asarray(ar2),
                         ft_scale=1.0 / HEADS)]

    # per-core featT [256, SLAB] fp16 in slot order
    featTs = []
    for c in range(NCORES):
        ft = np.zeros((256, NPOS * 128), np.float16)
        mask = core_of == c
        ids = np.arange(N_NODES)[mask]
        cols = pos_of[ids] * 128 + slot_of[ids]
        ft[:, cols] = features[ids].T.astype(np.float16)
        featTs.append(ft)

    nc = _build_program(J_a, J_b, colbase, ACH, BCH, ORDER)
    ins = [{"featT": featTs[c], "idxT": idxT[c],
            "W0e": Wes[0], "W1e": Wes[1], "W2e": Wes[2]}
           for c in range(NCORES)]
    res = bass_utils.run_bass_kernel_spmd(
        nc, ins, core_ids=list(range(NCORES)),
        trace=bool(os.environ.get("KERNEL_TRACE")))
    if os.environ.get("KERNEL_TRACE"):
        print("HW exec time:", res.exec_time_ns, "ns")
        kernel.last_exec_ns = res.exec_time_ns
        kernel.last_trace = res.instructions_and_trace

    out = np.empty((N_NODES, NCLS), np.float32)
    for c in range(NCORES):
        mask = core_of == c
        ids = np.arange(N_NODES)[mask]
        rows = pos_of[ids] * 128 + slot_of[ids]
        out[ids] = res.results[c]["out"][rows]
    return out



# revision 26
# speedup vs baseline: 1.2994x; 1.0181x over previous
"""GAT 3-layer kernel for 8 TRN2 NeuronCores (slot-major edge-parallel design).

Sharding: dst nodes packed into 392 blocks of 128 slots (one slot = one SBUF
partition), blocks dealt to 8 cores x 49 positions. Edges live on the free dim
of each block (j-slabs), split lo/hi by source-table row (<32768 vs >=) so the
int16 dma_gather indices stay in range. Node features/tables are all-gathered
per layer (chunked, overlapped with the node phase); everything else is local.
"""
import numpy as np

N_NODES = 50000
E_EDGES = 800000
IN_FEATS = 256
HID = 64
HEADS = 4
NCLS = 40
NEG_SLOPE = 0.2

NCORES = 8
NPOS = 49                    # blocks per core
NPA = 32                     # positions in table A (rest in table B)
SLAB_A = NPA * 128           # 4096 rows/core in table A
SLAB_B = (NPOS - NPA) * 128  # 2176 rows/core in table B
ROWS_A = NCORES * SLAB_A     # 32768
ROWS_B = NCORES * SLAB_B     # 17408
A_SIZE = ROWS_A - 8          # A-region real-node capacity (8 reserved slots)
DUMMY_A_ROW = ROWS_A - 1     # (core 7, pos 31, slot 127)
DUMMY_B_ROW = ROWS_B - 1     # (core 7, pos 48, slot 127)
RES_POS = (NPA - 1, NPOS - 1)  # slot 127 reserved on every core

def _srow_of(core, pos, slot, ach, bch):
    """Table row for a node at (core, pos, slot) under chunk-major layout."""
    pos = np.asarray(pos)
    out = np.zeros(pos.shape, np.int64)
    base = 0
    for (lo, hi) in ach:
        nrow = (hi - lo) * 128
        m = (pos >= lo) & (pos < hi)
        out[m] = base + core[m] * nrow + (pos[m] - lo) * 128 + slot[m]
        base += NCORES * nrow
    base = 0
    for (lo, hi) in bch:
        nrow = (hi - lo) * 128
        m = (pos >= lo) & (pos < hi)
        out[m] = base + core[m] * nrow + (pos[m] - lo) * 128 + slot[m]
        base += NCORES * nrow
    return out


def _reorder_positions(src, dst, core_of, pos_of, slot_of):
    """Relabel positions so process order (=label order per region) is a
    light-heavy-light pyramid: early labels are light (their AG chunks fire
    early in the sweep), heavies sit mid-sweep, and the final label of each
    region is light so the tail AllGather gating the next sweep is tiny.
    Returns (new pos_of, ACH, BCH, ORDER)."""
    in_b_node = pos_of >= NPA
    na = np.zeros(NPOS, np.int64)
    nb = np.zeros(NPOS, np.int64)
    # per-(core,pos,slot) counts of A/B edges -> per-pos max (the J profile)
    eb = (pos_of[src] >= NPA).astype(np.int64)
    key = (core_of[dst] * NPOS + pos_of[dst]) * 128 + slot_of[dst]
    for v, arr in ((0, na), (1, nb)):
        m = eb == v
        cnt = np.bincount(key[m], minlength=NCORES * NPOS * 128)
        mx = cnt.reshape(NCORES, NPOS, 128).max(axis=(0, 2))
        arr[:] = mx
    jt = np.maximum(na, 1) + np.maximum(nb, 1)

    def pyramid(labels):
        asc = sorted(labels, key=lambda p: jt[p])
        return asc[0::2] + asc[1::2][::-1]

    seqA = pyramid(list(range(NPA)))          # old labels, process order
    seqB = pyramid(list(range(NPA, NPOS)))
    # relabel: process order becomes label order within each region
    newpos = np.empty(NPOS, np.int64)
    for i, op in enumerate(seqA):
        newpos[op] = i
    for i, op in enumerate(seqB):
        newpos[op] = NPA + i
    pos_of2 = newpos[pos_of]
    jtA = jt[seqA]
    jtB = jt[seqB]

    def chunks(jts, lo0, fracs):
        cum = np.cumsum(jts) / jts.sum()
        n = len(jts)
        bnds = [lo0]
        for f in fracs:
            b = lo0 + int(np.searchsorted(cum, f) + 1)
            if b > bnds[-1] and b < lo0 + n - 1:
                bnds.append(b)
        if bnds[-1] != lo0 + n - 1:
            bnds.append(lo0 + n - 1)      # last chunk = single light position
        bnds.append(lo0 + n)
        return [(bnds[i], bnds[i + 1]) for i in range(len(bnds) - 1)]

    ach = chunks(jtA, 0, [0.33, 0.62, 0.86])
    bch = chunks(jtB, NPA, [0.5, 0.84])
    # merged process order: interleave A and B labels by cumulative fraction
    cumA = np.cumsum(jtA) / jtA.sum()
    cumB = np.cumsum(jtB) / jtB.sum()
    tagged = [(cumA[i], i) for i in range(NPA)] + \
             [(cumB[i], NPA + i) for i in range(NPOS - NPA)]
    order = [p for _, p in sorted(tagged)]
    return pos_of2, ach, bch, order


def _pack_nodes(src, dst):
    """Assign each node a (core, pos, slot); A-set = ids < A_SIZE -> pos < NPA.

    2D-banded packing: band by one in-degree coordinate, sort by the other
    within each band, so each 1024-node position block is homogeneous in
    (n_a, n_b) and per-block maxima (the slab padding) stay small."""
    deg = np.bincount(dst, minlength=N_NODES)
    n_a = np.bincount(dst[src < A_SIZE], minlength=N_NODES)
    n_b = deg - n_a

    ids = np.arange(N_NODES)
    a_ids = ids[:A_SIZE]
    b_ids = ids[A_SIZE:]

    def banded(idset, pri, sec, G):
        o = idset[np.argsort(-pri[idset], kind='stable')]
        out = []
        for i in range(0, len(o), 1024 * G):
            band = o[i:i + 1024 * G]
            out.append(band[np.argsort(-sec[band], kind='stable')])
        return np.concatenate(out)

    def chunk_cost(order, npos):
        tot, k = 0, 0
        for p in range(npos):
            take = order[k:k + 1024]
            k += len(take)
            tot += max(n_a[take].max(), 1) + max(n_b[take].max(), 1)
        return tot

    def best(idset, npos):
        cands = []
        for G in (2, 3, 4, 5, 6):
            for pri, sec in ((n_a, n_b), (n_b, n_a)):
                o = banded(idset, pri, sec, G)
                cands.append((chunk_cost(o, npos), o))
        cands.sort(key=lambda t: t[0])
        return cands[0][1]

    a_sorted = best(a_ids, NPA)
    b_sorted = best(b_ids, NPOS - NPA)

    core_of = np.full(N_NODES, -1, np.int32)
    pos_of = np.full(N_NODES, -1, np.int32)
    slot_of = np.full(N_NODES, -1, np.int32)

    def fill(sorted_ids, pos0):
        # snake deal: alternate core direction per position so the
        # degree-sorted blocks spread evenly across cores
        k = 0
        pos, ci = pos0, 0
        while k < len(sorted_ids):
            take = sorted_ids[k:k + 128]
            k += len(take)
            c = ci if (pos % 2 == 0) else NCORES - 1 - ci
            core_of[take] = c
            pos_of[take] = pos
            slot_of[take] = np.arange(len(take))
            ci += 1
            if ci == NCORES:
                ci = 0
                pos += 1

    fill(a_sorted, 0)
    fill(b_sorted, NPA)
    return core_of, pos_of, slot_of


def _evict_reserved(core_of, pos_of, slot_of):
    """After relabeling, slot 127 of the RES_POS positions (labels NPA-1 and
    NPOS-1, every core) must be node-free: the node phase stamps the dummy
    el marker (-80) on partition 127 there. Move any occupants to free
    slots elsewhere in their region."""
    occ = np.zeros((NCORES, NPOS, 128), bool)
    occ[core_of, pos_of, slot_of] = True
    for rp, lo, hi in ((NPA - 1, 0, NPA), (NPOS - 1, NPA, NPOS)):
        offenders = np.where((pos_of == rp) & (slot_of == 127))[0]
        free = np.argwhere(~occ[:, lo:hi, :])
        free = [(c, lo + p, s) for c, p, s in free
                if not (p + lo == rp and s == 127)]
        assert len(free) >= len(offenders)
        for i, node in enumerate(offenders):
            c, p, s = free[i]
            occ[core_of[node], pos_of[node], slot_of[node]] = False
            core_of[node], pos_of[node], slot_of[node] = c, p, s
            occ[c, p, s] = True


def _build_grids(src, dst, core_of, pos_of, slot_of, ach, bch):
    """Per-core wrapped int16 idx arrays + per-position J_a/J_b schedules."""
    in_b = (pos_of[src] >= NPA).astype(np.int64)
    srow = _srow_of(core_of[src], pos_of[src], slot_of[src], ach, bch)
    dcore = core_of[dst]
    dpos = pos_of[dst]
    dslot = slot_of[dst]
    drow = dcore * (NPOS * 128) + dpos * 128 + dslot

    order = np.lexsort((srow, in_b, drow))
    gkey = drow[order] * 2 + in_b[order]
    newgrp = np.concatenate([[True], gkey[1:] != gkey[:-1]])
    gstart = np.maximum.accumulate(np.where(newgrp, np.arange(E_EDGES), 0))
    rank = np.arange(E_EDGES) - gstart
    j_in = np.empty(E_EDGES, np.int64)
    j_in[order] = rank

    na_e = np.where(in_b == 0, j_in + 1, 0)
    nb_e = np.where(in_b == 1, j_in + 1, 0)
    J_a = np.zeros(NPOS, np.int64)
    J_b = np.zeros(NPOS, np.int64)
    np.maximum.at(J_a, dpos, na_e)
    np.maximum.at(J_b, dpos, nb_e)
    J_a = np.maximum(J_a, 1)
    J_b = np.maximum(J_b, 1)

    grids = [[np.concatenate([
                np.full((J_a[p], 128), DUMMY_A_ROW, np.int64),
                np.full((J_b[p], 128), DUMMY_B_ROW, np.int64)])
              for p in range(NPOS)] for c in range(NCORES)]
    j_eff = np.where(in_b == 0, j_in, J_a[dpos] + j_in)
    for e in range(E_EDGES):
        grids[dcore[e]][dpos[e]][j_eff[e], dslot[e]] = srow[e]

    def wrap(grid):
        flat = grid.reshape(-1)
        w = flat.reshape(-1, 8, 16).transpose(2, 0, 1).reshape(16, -1)
        return np.tile(w, (8, 1)).astype(np.int16)

    idxT = []
    colbase = []
    for c in range(NCORES):
        parts = []
        cb = []
        col = 0
        for p in range(NPOS):
            a_w = wrap(grids[c][p][:J_a[p]])
            b_w = wrap(grids[c][p][J_a[p]:])
            cb.append((col, col + a_w.shape[1]))
            col += a_w.shape[1] + b_w.shape[1]
            parts.append(a_w)
            parts.append(b_w)
        idxT.append(np.concatenate(parts, axis=1))
        colbase = cb
    return idxT, J_a.tolist(), J_b.tolist(), colbase


def _fold_weights(W, al, ar, ft_scale=1.0):
    """[W | A | B] with A[k,h]=sum_d W[k,h*D+d]*al[h,d] (el), B likewise (er).
    ft_scale scales only the W (message) part — used to fold the final
    mean-over-heads 1/H into layer 2's ft."""
    H, D = al.shape
    Wr = W.reshape(W.shape[0], H, D)
    A = np.einsum('khd,hd->kh', Wr, al)
    B = np.einsum('khd,hd->kh', Wr, ar)
    return np.concatenate([W * ft_scale, A, B], axis=1).astype(np.float16)


def _build_program(J_a, J_b, colbase, ACH, BCH, ORDER):
    import concourse.bacc as bacc
    import concourse.bass as bass
    import concourse.mybir as mybir
    from concourse.tile import TileContext

    f16 = mybir.dt.float16
    f32 = mybir.dt.float32
    AF = mybir.ActivationFunctionType
    OP = mybir.AluOpType

    TCOLS = [384, 384, 256]      # fp16 cols per table row per layer (stride)
    GCOLS = [256, 256, 160]      # ft cols per layer
    CCOLS = [264, 264, 168]      # node-phase out cols (ft + el4 + er4)
    TCC = [264, 264, 168]        # compact staged cols (ft + el4-as-f32)
    TOTC = colbase[-1][1] + J_b[-1] * 8
    Jt = [J_a[p] + J_b[p] for p in range(NPOS)]

    # Heavy positions are split into parts of <= MAXJ slabs so every g tile
    # comes from one uniform deep ring (6 bufs): small tiles keep SBUF in
    # budget while the deep ring keeps all 4 gather queues fed ahead of the
    # vector engine.
    MAXJ = 24
    GS_BUFS = 6
    PARTS = []
    for p in range(NPOS):
        if Jt[p] <= MAXJ:
            PARTS.append([(0, Jt[p])])
        else:
            m = Jt[p] // 2
            PARTS.append([(0, m), (m, Jt[p])])

    nc = bacc.Bacc("TRN2", num_devices=NCORES, num_swdge_queues=4)
    featT = nc.dram_tensor("featT", [256, NPOS * 128], f16, kind="ExternalInput")
    idxT = nc.dram_tensor("idxT", [128, TOTC], mybir.dt.int16, kind="ExternalInput")
    Wes = [nc.dram_tensor(f"W{l}e", [256, CCOLS[l]], f16, kind="ExternalInput")
           for l in range(3)]
    out_d = nc.dram_tensor("out", [NPOS * 128, NCLS], f32, kind="ExternalOutput")

    agins = []   # per layer: list of (pos_lo, pos_hi, is_b, row_base, tensor)
    for l in range(3):
        chunks = []
        base = 0
        for (lo, hi) in ACH:
            t = nc.dram_tensor(f"aga{l}_{lo}", [(hi - lo) * 128, TCOLS[l]],
                               f16, kind="Internal")
            chunks.append((lo, hi, False, base, t))
            base += NCORES * (hi - lo) * 128
        base = 0
        for (lo, hi) in BCH:
            t = nc.dram_tensor(f"agb{l}_{lo}", [(hi - lo) * 128, TCOLS[l]],
                               f16, kind="Internal")
            chunks.append((lo, hi, True, base, t))
            base += NCORES * (hi - lo) * 128
        agins.append(chunks)
    table_a = [nc.dram_tensor(f"tablea{l}", [ROWS_A, TCOLS[l]], f16,
                              kind="Internal", addr_space="Shared")
               for l in range(3)]
    table_b = [nc.dram_tensor(f"tableb{l}", [ROWS_B, TCOLS[l]], f16,
                              kind="Internal", addr_space="Shared")
               for l in range(3)]

    qn = [0]

    def next_q():
        qn[0] = (qn[0] + 1) % 4
        return qn[0]

    with TileContext(nc) as tc:
        with tc.tile_pool(name="resident", bufs=1) as rp, \
             tc.tile_pool(name="work", bufs=4) as wp, \
             tc.tile_pool(name="gather", bufs=2) as gp, \
             tc.tile_pool(name="nps", bufs=3, space="PSUM") as nps, \
             tc.tile_pool(name="tps", bufs=4, space="PSUM") as tps:

            ia = rp.tile([128, TOTC], mybir.dt.int16)
            nc.sync.dma_start(ia[:], idxT[:])
            NP128 = NPOS * 128
            hTbig = rp.tile([128, 2 * NP128], f16)
            nc.sync.dma_start(hTbig[:, :NP128], featT[0:128, :])
            nc.sync.dma_start(hTbig[:, NP128:], featT[128:256, :])
            hT = [[bass.AP(hTbig.tensor,
                           hTbig[:].offset + k * NP128 + p * 128,
                           [hTbig[:].ap[0], [1, 128]])
                   for p in range(NPOS)] for k in range(2)]
            er_own = [rp.tile([128, 4], f32, tag=f"er{p}", name=f"er{p}")
                      for p in range(NPOS)]
            Wt = [rp.tile([128, 2, CCOLS[l]], f16, tag=f"Wt{l}", name=f"Wt{l}")
                  for l in range(3)]
            for l in range(3):
                nc.sync.dma_start(
                    Wt[l][:], Wes[l][:].rearrange("(k p) n -> p k n", k=2))
            # fp32 identity for PE transpose
            colv = rp.tile([128, 128], mybir.dt.int32)
            nc.gpsimd.iota(colv[:], [[1, 128]], base=0, channel_multiplier=0)
            rowv = rp.tile([128, 1], mybir.dt.int32)
            nc.gpsimd.iota(rowv[:], [[0, 1]], base=0, channel_multiplier=1)
            row_b = bass.AP(rowv.tensor, rowv[:].offset,
                            [rowv[:].ap[0], [0, 128]])
            identf = rp.tile([128, 128], f32)
            nc.vector.tensor_tensor(identf[:], colv[:], row_b, OP.is_equal)
            # -80 at partition 127, 0 elsewhere (dummy-row el marker)
            dmask = rp.tile([128, 1], f32)
            nc.vector.tensor_scalar(dmask[:], rowv[:], 127, -80.0,
                                    OP.is_equal, OP.mult)
            # const tiles for contention-free tensor_tensor broadcasts
            ones_t = rp.tile([128, 1], f32)
            nc.vector.memset(ones_t[:], 1.0)
            negt = rp.tile([128, 1], f32)
            nc.vector.memset(negt[:], -1.0)

            def node_phase(l, p):
                # copies/casts run on the scalar engine: DVE copy/cast ops
                # enter 2-port perf mode and fully block GpSimd SWDGE
                # descriptor generation (ACT never contends).
                GC, CC = GCOLS[l], CCOLS[l]
                ps = nps.tile([128, CC], f32, tag="nodeps")
                for k in range(2):
                    nc.tensor.matmul(
                        ps[:], hT[k][p],
                        Wt[l][:].rearrange("p k n -> k p n")[k],
                        start=(k == 0), stop=(k == 1))
                one4 = bass.AP(ones_t.tensor, ones_t[:].offset,
                               [ones_t[:].ap[0], [0, 4]])
                nc.vector.tensor_tensor(er_own[p][:], ps[:, GC + 4:GC + 8],
                                        one4, OP.mult)
                stage = wp.tile([128, TCOLS[l]], f16, tag="stage")
                oneGC = bass.AP(ones_t.tensor, ones_t[:].offset,
                                [ones_t[:].ap[0], [0, GC]])
                nc.vector.tensor_tensor(stage[:, :GC], ps[:, :GC], oneGC,
                                        OP.mult)
                st32 = stage[:].bitcast(f32)
                if p in RES_POS:
                    dm_b = bass.AP(dmask.tensor, dmask[:].offset,
                                   [dmask[:].ap[0], [0, 4]])
                    nc.vector.tensor_tensor(st32[:, GC // 2:GC // 2 + 4],
                                            ps[:, GC:GC + 4], dm_b, OP.add)
                else:
                    nc.vector.tensor_tensor(st32[:, GC // 2:GC // 2 + 4],
                                            ps[:, GC:GC + 4], one4, OP.mult)
                # stage -> the agin chunk containing position p (scalar-queue
                # HWDGE so the sync queue stays free)
                for (lo, hi, is_b, rb, t) in agins[l]:
                    if lo <= p < hi:
                        nc.scalar.dma_start(
                            t[(p - lo) * 128:(p - lo + 1) * 128, :], stage[:])
                # fire the collective for any chunk that just completed.
                # Fine-grained chunks keep each trigger's stage-DMA wait
                # short, so the in-order GpSimd queue is only briefly
                # blocked; the tiny last chunk keeps the AG that gates the
                # next sweep's gathers off the critical path.
                for (lo, hi, is_b, rb, t) in agins[l]:
                    if p == hi - 1:
                        tab = table_b[l] if is_b else table_a[l]
                        nrow = (hi - lo) * 128
                        nc.gpsimd.collective_compute(
                            "AllGather", OP.bypass,
                            replica_groups=[list(range(NCORES))],
                            ins=[t[:].opt()],
                            outs=[tab[rb:rb + NCORES * nrow, :].opt()])

            g_tiles = {}

            def edge_gather(l, p):
                # each part -> its own tile; pieces split across the 4 SWDGE
                # queues (gather throughput is per-row per-queue)
                GC, TC = GCOLS[l], TCOLS[l]
                Jl, Jh = J_a[p], J_b[p]
                locol, hicol = colbase[p]
                tiles = []
                for (j0, j1) in PARTS[p]:
                    nj = j1 - j0
                    g = gp.tile([128, nj, 384], f16, tag="gs", bufs=GS_BUFS,
                                name="g")
                    tiles.append(g)

                    def gg(table, col0, nj2, dj):
                        # sub-split big pieces across two queues
                        if nj2 >= 6:
                            h = nj2 // 2
                            pieces = [(0, h), (h, nj2)]
                        else:
                            pieces = [(0, nj2)]
                        for (a, b) in pieces:
                            nc.gpsimd.dma_gather(
                                bass.AP(g.tensor,
                                        g[:].offset + (dj + a) * TC,
                                        [g[:].ap[0], [TC, b - a], [1, TC]]),
                                table[:, :],
                                ia[:, col0 + a * 8:col0 + b * 8],
                                (b - a) * 128, (b - a) * 128, TC,
                                single_packet=False, queue_num=next_q())

                    # A-piece of this part: slabs [j0, min(j1, Jl))
                    if j0 < Jl:
                        s0, s1 = j0, min(j1, Jl)
                        gg(table_a[l], locol + s0 * 8, s1 - s0, 0)
                    # B-piece: slabs [max(j0, Jl), j1)
                    if j1 > Jl:
                        s0, s1 = max(j0, Jl), j1
                        gg(table_b[l], hicol + (s0 - Jl) * 8, s1 - s0,
                           s0 - j0)
                g_tiles[p] = tiles

            def edge_phase(l, p):
                GC, TC = GCOLS[l], TCOLS[l]
                tiles = g_tiles.pop(p)
                parts = PARTS[p]
                # per part: e = el + er; ex = exp(lrelu(e)) on scalar;
                # den_part = sum_j ex
                exs, dens = [], []
                for g, (j0, j1) in zip(tiles, parts):
                    nj = j1 - j0
                    elv = bass.AP(g.tensor, g[:].offset,
                                  [g[:].ap[0], [TC, nj], [1, TC]]
                                  ).bitcast(f32)
                    el_hm = bass.AP(elv.tensor, elv.offset + GC // 2,
                                    [elv.ap[0], [1, 4], [TC // 2, nj]])
                    e_t = wp.tile([128, 4, nj], f32, tag="e")
                    er_b = bass.AP(er_own[p].tensor, er_own[p][:].offset,
                                   [er_own[p][:].ap[0], [1, 4], [0, nj]])
                    nc.vector.tensor_tensor(e_t[:], el_hm, er_b, OP.add)
                    ex1 = wp.tile([128, 4, nj], f32, tag="ex1")
                    nc.scalar.activation(ex1[:], e_t[:], AF.Exp)
                    ex2 = wp.tile([128, 4, nj], f32, tag="ex2")
                    nc.scalar.activation(ex2[:], e_t[:], AF.Exp,
                                         scale=NEG_SLOPE)
                    nc.vector.tensor_tensor(ex1[:], ex1[:], ex2[:], OP.max)
                    den = wp.tile([128, 4], f32, tag="den")
                    nc.vector.tensor_reduce(den[:, :, None], ex1[:],
                                            op=OP.add,
                                            axis=mybir.AxisListType.X)
                    exs.append(ex1)
                    dens.append(den)
                if len(dens) > 1:
                    nc.vector.tensor_tensor(dens[0][:], dens[0][:],
                                            dens[1][:], OP.add)
                rd = wp.tile([128, 4], f32, tag="rd")
                nc.vector.reciprocal(rd[:], dens[0][:])
                # per part: alpha = ex*rd (fp16); msg = alpha*ft in place;
                # tree-sum msg over j -> ro (accumulated across parts)
                D = GC // 4
                ro = wp.tile([128, GC], f32, tag="ro")
                for pi, (g, (j0, j1)) in enumerate(zip(tiles, parts)):
                    nj = j1 - j0
                    ex1 = exs[pi]
                    alpha = wp.tile([128, 4, nj], f16, tag="alpha")
                    rd_b = bass.AP(rd.tensor, rd[:].offset,
                                   [rd[:].ap[0], [1, 4], [0, nj]])
                    nc.vector.tensor_tensor(alpha[:], ex1[:], rd_b, OP.mult)
                    al_b = bass.AP(alpha.tensor, alpha[:].offset,
                                   [alpha[:].ap[0], [1, nj], [nj, 4], [0, D]])
                    ft4 = bass.AP(g.tensor, g[:].offset,
                                  [g[:].ap[0], [TC, nj], [D, 4], [1, D]])
                    nc.vector.tensor_tensor(ft4, ft4, al_b, OP.mult)

                    def jsl(j0c, cnt):
                        return bass.AP(g.tensor, g[:].offset + j0c * TC,
                                       [g[:].ap[0], [TC, cnt], [1, GC]])

                    n = nj
                    while n > 2:
                        h = n // 2
                        nc.vector.tensor_tensor(jsl(0, h), jsl(0, h),
                                                jsl(n - h, h), OP.add)
                        n -= h
                    assert n == 2, nj
                    if pi == 0:
                        nc.vector.tensor_tensor(ro[:], jsl(0, 1),
                                                jsl(1, 1), OP.add)
                    else:
                        nc.vector.tensor_tensor(jsl(0, 1), jsl(0, 1),
                                                jsl(1, 1), OP.add)
                        nc.vector.tensor_tensor(ro[:], ro[:], jsl(0, 1),
                                                OP.add)
                if l < 2:
                    # h = elu(ro) = max(ro, min(exp(ro),1)-1); -> fp16 -> hT
                    # (tensor_tensor only — DVE tensor_scalar would block
                    # GpSimd descriptor generation)
                    ev = wp.tile([128, GC], f32, tag="ev")
                    nc.scalar.activation(ev[:], ro[:], AF.Exp)
                    one_b = bass.AP(ones_t.tensor, ones_t[:].offset,
                                    [ones_t[:].ap[0], [0, GC]])
                    nc.vector.tensor_tensor(ev[:], ev[:], one_b, OP.min)
                    nc.vector.tensor_tensor(ev[:], ev[:], one_b, OP.subtract)
                    nc.vector.tensor_tensor(ro[:], ro[:], ev[:], OP.max)
                    for k in range(2):
                        tp = tps.tile([128, 128], f32, tag="trps")
                        nc.tensor.transpose(
                            tp[:], ro[:, k * 128:(k + 1) * 128], identf[:])
                        one128 = bass.AP(ones_t.tensor, ones_t[:].offset,
                                         [ones_t[:].ap[0], [0, 128]])
                        nc.vector.tensor_tensor(hT[k][p], tp[:], one128,
                                                OP.mult)
                else:
                    # logits = mean over heads (1/H folded into W2e ft);
                    # log_softmax
                    z = wp.tile([128, NCLS], f32, tag="z")
                    ro_h = bass.AP(ro.tensor, ro[:].offset,
                                   [ro[:].ap[0], [1, NCLS], [NCLS, 4]])
                    z_v = bass.AP(z.tensor, z[:].offset,
                                  [z[:].ap[0], [1, NCLS], [0, 1]])
                    nc.vector.tensor_reduce(z_v, ro_h, op=OP.add,
                                            axis=mybir.AxisListType.X)
                    m = wp.tile([128, 1], f32, tag="m")
                    nc.vector.tensor_reduce(m[:], z[:], op=OP.max,
                                            axis=mybir.AxisListType.X)
                    nm = wp.tile([128, 1], f32, tag="nm")
                    nc.vector.tensor_tensor(nm[:], m[:], negt[:], OP.mult)
                    ez = wp.tile([128, NCLS], f32, tag="ez")
                    s = wp.tile([128, 1], f32, tag="s")
                    nc.scalar.activation(ez[:], z[:], AF.Exp, bias=nm[:],
                                         accum_out=s[:])
                    lns = wp.tile([128, 1], f32, tag="lns")
                    nc.scalar.activation(lns[:], s[:], AF.Ln)
                    b = wp.tile([128, 1], f32, tag="b")
                    nc.vector.tensor_tensor(b[:], m[:], lns[:], OP.add)
                    lp = wp.tile([128, NCLS], f32, tag="lp")
                    b_b = bass.AP(b.tensor, b[:].offset,
                                  [b[:].ap[0], [0, NCLS]])
                    nc.vector.tensor_tensor(lp[:], z[:], b_b, OP.subtract)
                    nc.sync.dma_start(out_d[p * 128:(p + 1) * 128, :], lp[:])

            # sweep 0: node phase of layer 0 (+ its chunked all-gathers)
            for p in ORDER:
                node_phase(0, p)
            # sweeps 1,2: edge(l-1) interleaved with node(l)
            for l in (1, 2):
                for p in ORDER:
                    edge_gather(l - 1, p)
                for p in ORDER:
                    edge_phase(l - 1, p)
                    node_phase(l, p)
            # sweep 3: edge phase of layer 2 -> logits
            for p in ORDER:
                edge_gather(2, p)
            for p in ORDER:
                edge_phase(2, p)
    nc.compile()
    return nc


def _install_trace_shim():
    """Provide antenv.axon_hooks (get/set NTFF profile hook) when absent."""
    import sys, types
    try:
        from antenv.axon_hooks import get_axon_ntff_profile_hook  # noqa
        return
    except ImportError:
        pass
    mod = types.ModuleType("antenv.axon_hooks")
    _hook = [None]
    mod.set_axon_ntff_profile_hook = lambda h: _hook.__setitem__(0, h)
    mod.get_axon_ntff_profile_hook = lambda: _hook[0]
    sys.modules["antenv.axon_hooks"] = mod
    import antenv
    antenv.axon_hooks = mod
    if "/root/.axon_site" not in sys.path:
        sys.path.insert(0, "/root/.axon_site")
    from trn_agent_boot.trn_boot import _ntff_profile_via_ctypes
    mod.set_axon_ntff_profile_hook(
        _ntff_profile_via_ctypes("/opt/axon/libaxon_pjrt.so"))


def kernel(features, src, dst, W0, al0, ar0, W1, al1, ar1, W2, al2, ar2):
    import sys, os
    for pth in ("/root/axon_fix", "/opt/trn_rl_repo"):
        if os.path.isdir(pth) and pth not in sys.path:
            sys.path.insert(0, pth)
    if os.environ.get("KERNEL_TRACE"):
        _install_trace_shim()
    from concourse import bass_utils

    src = np.asarray(src).astype(np.int64)
    dst = np.asarray(dst).astype(np.int64)
    features = np.asarray(features, np.float32)

    core_of, pos_of, slot_of = _pack_nodes(src, dst)
    pos_of, ACH, BCH, ORDER = _reorder_positions(src, dst, core_of, pos_of,
                                                 slot_of)
    _evict_reserved(core_of, pos_of, slot_of)
    idxT, J_a, J_b, colbase = _build_grids(src, dst, core_of, pos_of, slot_of,
                                           ACH, BCH)
    Wes = [_fold_weights(np.asarray(W0, np.float32), np.asarray(al0), np.asarray(ar0)),
           _fold_weights(np.asarray(W1, np.float32), np.asarray(al1), np.asarray(ar1)),
           _fold_weights(np.asarray(W2, np.float32), np.asarray(al2), np.asarray(ar2),
                         ft_scale=1.0 / HEADS)]

    # per-core featT [256, SLAB] fp16 in slot order
    featTs = []
    for c in range(NCORES):
        ft = np.zeros((256, NPOS * 128), np.float16)
        mask = core_of == c
        ids = np.arange(N_NODES)[mask]
        cols = pos_of[ids] * 128 + slot_of[ids]
        ft[:, cols] = features[ids].T.astype(np.float16)
        featTs.append(ft)

    nc = _build_program(J_a, J_b, colbase, ACH, BCH, ORDER)
    ins = [{"featT": featTs[c], "idxT": idxT[c],
            "W0e": Wes[0], "W1e": Wes[1], "W2e": Wes[2]}
           for c in range(NCORES)]
    res = bass_utils.run_bass_kernel_spmd(
        nc, ins, core_ids=list(range(NCORES)),
        trace=bool(os.environ.get("KERNEL_TRACE")))
    if os.environ.get("KERNEL_TRACE"):
        print("HW exec time:", res.exec_time_ns, "ns")
        kernel.last_exec_ns = res.exec_time_ns
        kernel.last_trace = res.instructions_and_trace

    out = np.empty((N_NODES, NCLS), np.float32)
    for c in range(NCORES):
        mask = core_of == c
        ids = np.arange(N_NODES)[mask]
        rows = pos_of[ids] * 128 + slot_of[ids]
        out[ids] = res.results[c]["out"][rows]
    return out

